# revision 3
# baseline (speedup 1.0000x reference)
"""Bass/Trainium2 kernel for nn_BuildLstmUnrollNet.

Problem: 2-layer LSTM, unrolled T=11 steps with per-step (non-shared)
weights, B=8192, R=425, IN=20.  Output block t is the last-layer h
*before* step t, so only steps 0..9 need computing.

Strategy (data-parallel over batch, 8 cores x 1024 rows):
  - Step 0 runs in bf16: its matmul operands are the *initial* states,
    which are unbounded N(0,1) draws -- fp8 there costs ~2.5e-2 rel err.
  - Steps 1..9 run the gate matmuls in fp8-e4m3 DoubleRow (both
    operands fp8, 256-deep contraction per pass): after step 0 every h
    is a tanh*sigmoid product bounded by 1, and e4m3 keeps the end-to-
    end rel err ~4e-3 (vs the 2e-2 gate).  DoubleRow halves both the
    pass count and the per-column cost.
  - States kept batch-major in ONE packed bf16 buffer per m-tile:
    cols [h0(425) | 1.0 | x(20) | h1(425) | 1.0 | pad] = 896 = 7*128.
    The second 1.0 (col 871) pairs with a host-precomputed fp8 residual
    weight row that cancels most of layer-1's bias quantization error.
  - Gates are computed batch-major in PSUM with transposed activations
    stationary (lhsT) and weights moving: layer 0 contracts packed rows
    0..511, layer 1 rows 0..1023 (chunk 7 is a zeroed pad chunk so
    layer 1 is exactly 4 DoubleRow passes).
  - The whole unroll is ONE software-pipelined stream of (step, layer,
    m-tile) stages: each stage issues its matmuls + the gate
    nonlinearities (sig/tanh) + c-update, while the *previous* stage's
    tanh(c) + h-write + DMAs are issued one stage later.  This keeps
    the Activation engine (the post-fp8 bottleneck) free of
    head-of-line stalls: its queue is ...sig(s) tg(s) ttc(s-1) sig(s+1)...
  - The recurrent transpose h -> hT bounces through DRAM so the x-bar
    DMA transpose runs on the SP/HWDGE queue; fp8 can't ride the 2-byte
    transpose path, so the GPSIMD/Pool engine (otherwise idle) converts
    the transposed bf16 chunks to fp8 right after each transpose group.
    h1 transposes for step t+1 are split: batch rows 0..511 are
    transposed during step t's layer-1 phase, rows 512..1023 at the top
    of step t+1.
  - Cell math all-bf16 (2x DVE mode); output DMA reads the packed h1
    slice (out tensor is bf16, upconverted on the host).

kernel(**inputs) takes full-size numpy inputs, packs/shards on the
host, runs the same program SPMD on cores 0..7, and reassembles the
full [8192, 4675] fp32 output (block 0 comes from the initial state).
"""

import numpy as np
import ml_dtypes

BF16 = ml_dtypes.bfloat16
FP8 = ml_dtypes.float8_e4m3

B = 8192
NCORES = 8
BC = B // NCORES          # batch rows per core (1024)
NB = BC // 128            # m-tiles per core (8)
R = 425
IN = 20
GN = 4 * R                # 1700 gate columns
H1OFF = R + 1 + IN        # 446: h1 col offset in the packed state block
HC = 896                  # packed state block width (7*128)
NKC = 7                   # bf16 transpose chunks
NK0 = 4                   # layer-0 K-chunks (rows 0..511)
NKT = 11                  # step-0 bf16 weight K-blocks (4 + 7)
NW8 = 12                  # fp8 weight K-blocks (4 + 8, block 11 zero)
NSTEPS = 10
# N chunks of the 1700-wide gate output (one PSUM bank each)
NCHUNKS = [(0, 512), (512, 512), (1024, 512), (1536, 164)]

# set by test.py to profile; results stashed in LAST_RESULT
TRACE = False
LAST_RESULT = None


def build_bass(n_steps=NSTEPS, finalize=True):
    import concourse.bacc as bacc
    import concourse.mybir as mybir
    import concourse.tile as tile

    f32 = mybir.dt.float32
    bf16 = mybir.dt.bfloat16
    fp8 = mybir.dt.float8e4
    Sig = mybir.ActivationFunctionType.Sigmoid
    Tanh = mybir.ActivationFunctionType.Tanh
    DR = mybir.MatmulPerfMode.DoubleRow

    nc = bacc.Bacc()

    n8 = max(n_steps - 1, 1)
    w0_d = nc.declare_dram_parameter("w0", [128, NKT * GN], bf16, False)
    w8_d = nc.declare_dram_parameter("w8", [n8, 128, NW8 * GN], fp8, False)
    hci_d = nc.declare_dram_parameter("hci", [128, NB * HC], bf16, False)
    htci_d = nc.declare_dram_parameter("htci", [128, NKC * BC], bf16, False)
    c0i_d = nc.declare_dram_parameter("c0i", [128, NB * R], bf16, False)
    c1i_d = nc.declare_dram_parameter("c1i", [128, NB * R], bf16, False)
    out_d = nc.declare_dram_parameter("out", [BC, n_steps * R], bf16, True)
    # DRAM bounce buffer for the recurrent transpose (batch-major packed h)
    hd = nc.dram_tensor("hd", [BC, HC], bf16)

    with tile.TileContext(nc) as tc:
        with (
            tc.tile_pool(name="consts", bufs=1) as consts,
            tc.tile_pool(name="wpool", bufs=2) as wpool,
            tc.tile_pool(name="gpsum", bufs=2, space="PSUM") as gpsum,
            tc.tile_pool(name="tmp", bufs=3) as tmp,
        ):
            # persistent state tiles
            hs_t = consts.tile([128, NB * HC], bf16)    # packed batch-major
            htc = consts.tile([128, NKC, BC], bf16)     # transposed (lhsT)
            htc8 = consts.tile([128, 8, BC], fp8)       # fp8 lhsT (DR)
            c0 = consts.tile([128, NB * R], bf16)
            c1 = consts.tile([128, NB * R], bf16)
            w0t = consts.tile([128, NKT * GN], bf16)    # step-0 weights

            # init DMAs on the SP (HWDGE) queue, most-urgent first, while
            # the weight tables stream on the Pool (SWDGE) queue
            for k in range(NKC):
                nc.sync.dma_start(htc[:, k, :], htci_d[:, k * BC:(k + 1) * BC])
            nc.sync.dma_start(c0[:], c0i_d[:])
            nc.sync.dma_start(hs_t[:], hci_d[:])
            nc.sync.dma_start(c1[:], c1i_d[:])

            # step-0 weights, split per k-block so matmuls start early
            for k in range(NKT):
                nc.gpsimd.dma_start(w0t[:, k * GN:(k + 1) * GN],
                                    w0_d[:, k * GN:(k + 1) * GN])
            # first fp8 table (for step 1)
            w8cur = None
            if n_steps > 1:
                w8cur = wpool.tile([128, NW8, GN], fp8, tag="w8")
                for c in range(4):
                    nc.gpsimd.dma_start(w8cur[:, 3 * c:3 * (c + 1), :],
                                        w8_d[0][:, 3 * c * GN:3 * (c + 1) * GN])
            # zero pad chunk for layer-1's 4th DoubleRow pass
            nc.gpsimd.memset(htc8[:, 7, :], 0.0)

            # PE warm-up: keep the p-state ramp busy while init DMAs land
            warm = consts.tile([128, 128], bf16)
            nc.vector.memset(warm[:], 0.0)
            wps = gpsum.tile([128, 512], f32, tag="g")
            for i in range(20):
                nc.tensor.matmul(wps[:, 0:128], warm[:], warm[:],
                                 start=True, stop=True)

            w8tab = {}           # step -> weight tile
            if n_steps > 1:
                w8tab[1] = w8cur

            def h1_xpose(half):
                """Transpose h1 rows (chunks 4..6) of one batch half from
                the DRAM bounce, then fp8-convert them on Pool."""
                rows = slice(half * 512, (half + 1) * 512)
                for k in range(NK0, NKC):
                    nc.sync.dma_start(
                        out=htc[:, k, half * 512:(half + 1) * 512],
                        in_=hd[rows, k * 128:(k + 1) * 128],
                        transpose=True)
                nc.gpsimd.tensor_copy(
                    htc8[:, NK0:NKC, half * 512:(half + 1) * 512],
                    htc[:, NK0:NKC, half * 512:(half + 1) * 512])

            def mm(t, layer, m):
                g = gpsum.tile([128, GN], f32, tag="g")
                if t == 0:
                    if layer == 0:
                        kplan = [(k, k) for k in range(NK0)]
                    else:
                        kplan = ([(k, NK0 + k) for k in range(NK0, NKC)]
                                 + [(k, NK0 + k) for k in range(NK0)])
                    nk = len(kplan)
                    for ki, (kk, wk) in enumerate(kplan):
                        lhsT = htc[:, kk, m * 128:(m + 1) * 128]
                        for (no, nw) in NCHUNKS:
                            nc.tensor.matmul(
                                g[:, no:no + nw], lhsT,
                                w0t[:, wk * GN + no:wk * GN + no + nw],
                                start=(ki == 0), stop=(ki == nk - 1))
                else:
                    w8t = w8tab[t]
                    jplan = [0, 1] if layer == 0 else [2, 3, 0, 1]
                    nj = len(jplan)
                    for ji, j in enumerate(jplan):
                        wb = 2 * j + (0 if layer == 0 else 4)
                        lhsT = htc8[:, 2 * j:2 * j + 2, m * 128:(m + 1) * 128]
                        for (no, nw) in NCHUNKS:
                            nc.tensor.matmul(
                                g[:, no:no + nw], lhsT,
                                w8t[:, wb:wb + 2, no:no + nw],
                                start=(ji == 0), stop=(ji == nj - 1),
                                perf_mode=DR)
                return g

            def cell_a(g, layer, m):
                """Gate nonlinearities + c update (issued in-stage)."""
                cst = c0 if layer == 0 else c1
                cs = cst[:, m * R:(m + 1) * R]
                tsig = tmp.tile([128, 3 * R], bf16, tag="tsig")
                nc.scalar.activation(tsig[:], g[:, 0:3 * R], Sig)
                tg = tmp.tile([128, R], bf16, tag="tg")
                nc.scalar.activation(tg[:], g[:, 3 * R:4 * R], Tanh)
                tfc = tmp.tile([128, R], bf16, tag="tfc")
                nc.vector.tensor_mul(tfc[:], tsig[:, R:2 * R], cs)
                tig = tmp.tile([128, R], bf16, tag="tig")
                nc.vector.tensor_mul(tig[:], tsig[:, 0:R], tg[:])
                nc.vector.tensor_add(cs, tfc[:], tig[:])
                return tsig

            def cell_b(tsig, t, layer, m):
                """tanh(c) + h write + recurrence DMAs (issued one stage
                later so the Activation queue never stalls on the DVE)."""
                cst = c0 if layer == 0 else c1
                cs = cst[:, m * R:(m + 1) * R]
                ttc = tmp.tile([128, R], bf16, tag="ttc")
                nc.scalar.activation(ttc[:], cs, Tanh)
                off = m * HC + (0 if layer == 0 else H1OFF)
                nc.vector.tensor_mul(hs_t[:, off:off + R],
                                     tsig[:, 2 * R:3 * R], ttc[:])
                if layer == 0:
                    # bounce h0' (+ consts); transpose+convert in 3 row-
                    # groups so layer 1's first m-tiles unblock early
                    nc.sync.dma_start(hd[m * 128:(m + 1) * 128, 0:512],
                                      hs_t[:, m * HC:m * HC + 512])
                    if m in (2, 5, 7):
                        lo = {2: 0, 5: 384, 7: 768}[m]
                        hi = {2: 384, 5: 768, 7: 1024}[m]
                        for k in range(NK0):
                            nc.sync.dma_start(
                                out=htc[:, k, lo:hi],
                                in_=hd[lo:hi, k * 128:(k + 1) * 128],
                                transpose=True)
                        nc.gpsimd.tensor_copy(htc8[:, 0:NK0, lo:hi],
                                              htc[:, 0:NK0, lo:hi])
                else:
                    nc.sync.dma_start(
                        out_d[m * 128:(m + 1) * 128, t * R:(t + 1) * R],
                        hs_t[:, m * HC + H1OFF:m * HC + H1OFF + R])
                    if t < n_steps - 1:
                        nc.sync.dma_start(hd[m * 128:(m + 1) * 128, 512:HC],
                                          hs_t[:, m * HC + 512:(m + 1) * HC])

            # ---- the software-pipelined stage stream ----
            stages = [(t, layer, m)
                      for t in range(n_steps)
                      for layer in range(2)
                      for m in range(NB)]
            pending = None        # (tsig, t, layer, m) awaiting cell_b
            for (t, layer, m) in stages:
                # pre-hooks (issued before this stage's matmuls)
                if layer == 0 and m == 0 and t >= 1:
                    if t + 1 < n_steps:
                        w8n = wpool.tile([128, NW8, GN], fp8, tag="w8")
                        for c in range(4):
                            nc.gpsimd.dma_start(
                                w8n[:, 3 * c:3 * (c + 1), :],
                                w8_d[t][:, 3 * c * GN:3 * (c + 1) * GN])
                        w8tab[t + 1] = w8n
                if layer == 0 and m == 1 and t >= 1:
                    # h1 rows of batch half 1 (bounced at the end of the
                    # previous step's layer 1)
                    h1_xpose(1)

                g = mm(t, layer, m)
                tsig = cell_a(g, layer, m)
                if pending is not None:
                    cell_b(*pending)
                pending = (tsig, t, layer, m)

                if layer == 1 and m == 6 and t < n_steps - 1:
                    # h1 rows of batch half 0 for the NEXT step: their
                    # bounces (m0..3) are already in DRAM; chunks 4..6
                    # cols 0..511 are no longer read this step
                    h1_xpose(0)
            cell_b(*pending)
    if finalize:
        nc.finalize()
    return nc


def _pack_pf(a):
    """[BC, C] -> [128, NB*C] with m-tile m at cols m*C."""
    c = a.shape[1]
    return np.ascontiguousarray(
        a.reshape(NB, 128, c).transpose(1, 0, 2).reshape(128, NB * c))


def _pack_kt(a):
    """[BC, HC] -> transposed [128, NKC*BC] with K-chunk k at cols k*BC."""
    return np.ascontiguousarray(
        a.T.reshape(NKC, 128, BC).transpose(1, 0, 2).reshape(128, NKC * BC))


def prep_inputs(x, init_states_input, W_i2h0, b_i2h0, W_h2h0, b_h2h0,
                W_i2h1, b_i2h1, W_h2h1, b_h2h1, n_steps=NSTEPS):
    """Host-side packing.  Returns (in_maps, h1_init_full)."""
    x = np.asarray(x, np.float32)
    init = np.asarray(init_states_input, np.float32)
    W_i2h0 = np.asarray(W_i2h0, np.float32)
    b_i2h0 = np.asarray(b_i2h0, np.float32)
    W_h2h0 = np.asarray(W_h2h0, np.float32)
    b_h2h0 = np.asarray(b_h2h0, np.float32)
    W_i2h1 = np.asarray(W_i2h1, np.float32)
    b_i2h1 = np.asarray(b_i2h1, np.float32)
    W_h2h1 = np.asarray(W_h2h1, np.float32)
    b_h2h1 = np.asarray(b_h2h1, np.float32)

    # step-0 bf16 weight table: K-major blocks, transposed to [K, 4R],
    # rows matching the packed state layout [h0 | 1 | x | h1 | 1 | pad]
    Wd0 = np.zeros((NKT * 128, GN), np.float32)
    Wd0[0:R] = W_h2h0[0].T
    Wd0[R] = b_i2h0[0] + b_h2h0[0]
    Wd0[R + 1:R + 1 + IN] = W_i2h0[0].T
    o = NK0 * 128
    Wd0[o:o + R] = W_i2h1[0].T
    Wd0[o + R] = b_i2h1[0] + b_h2h1[0]
    Wd0[o + H1OFF:o + H1OFF + R] = W_h2h1[0].T
    w0_dev = np.ascontiguousarray(
        Wd0.reshape(NKT, 128, GN).transpose(1, 0, 2)
        .reshape(128, NKT * GN)).astype(BF16)

    # fp8 tables for steps 1..n-1: blocks 0..3 layer 0 (512 rows),
    # blocks 4..11 layer 1 (1024 rows incl. zero pad + bias residual)
    n8 = max(n_steps - 1, 1)
    Wd8 = np.zeros((n8, NW8 * 128, GN), np.float32)
    for t in range(1, n_steps):
        d = Wd8[t - 1]
        d[0:R] = W_h2h0[t].T
        d[R] = b_i2h0[t] + b_h2h0[t]
        d[R + 1:R + 1 + IN] = W_i2h0[t].T
        o = NK0 * 128
        d[o:o + R] = W_i2h1[t].T
        b1 = b_i2h1[t] + b_h2h1[t]
        d[o + R] = b1
        d[o + H1OFF:o + H1OFF + R] = W_h2h1[t].T
        # residual row (pairs with the 1.0 at packed col 871): cancels
        # most of the fp8 quantization error of the layer-1 bias row
        d[o + H1OFF + R] = b1 - b1.astype(FP8).astype(np.float32)
    w8_dev = np.ascontiguousarray(
        Wd8.reshape(n8, NW8, 128, GN).transpose(0, 2, 1, 3)
        .reshape(n8, 128, NW8 * GN)).astype(FP8)

    init4 = init.reshape(B, 4, R)
    h0_full, c0_full = init4[:, 0], init4[:, 1]
    h1_full, c1_full = init4[:, 2], init4[:, 3]

    in_maps = []
    for c in range(NCORES):
        sl = slice(c * BC, (c + 1) * BC)
        hcp = np.zeros((BC, HC), np.float32)
        hcp[:, 0:R] = h0_full[sl]
        hcp[:, R] = 1.0
        hcp[:, R + 1:R + 1 + IN] = x[sl]
        hcp[:, H1OFF:H1OFF + R] = h1_full[sl]
        hcp[:, H1OFF + R] = 1.0
        hcp = hcp.astype(BF16)
        in_maps.append({
            "w0": w0_dev,
            "w8": w8_dev,
            "hci": _pack_pf(hcp),
            "htci": _pack_kt(hcp),
            "c0i": _pack_pf(np.ascontiguousarray(c0_full[sl])).astype(BF16),
            "c1i": _pack_pf(np.ascontiguousarray(c1_full[sl])).astype(BF16),
        })
    return in_maps, h1_full


def kernel(x, init_states_input, W_i2h0, b_i2h0, W_h2h0, b_h2h0,
           W_i2h1, b_i2h1, W_h2h1, b_h2h1):
    global LAST_RESULT
    from concourse.bass_utils import run_bass_kernel_spmd

    in_maps, h1_full = prep_inputs(
        x, init_states_input, W_i2h0, b_i2h0, W_h2h0, b_h2h0,
        W_i2h1, b_i2h1, W_h2h1, b_h2h1)

    nc = build_bass(NSTEPS)
    res = run_bass_kernel_spmd(nc, in_maps, list(range(NCORES)), trace=TRACE)
    LAST_RESULT = res

    out = np.empty((B, (NSTEPS + 1) * R), np.float32)
    out[:, 0:R] = h1_full
    for c in range(NCORES):
        out[c * BC:(c + 1) * BC, R:] = res.results[c]["out"].astype(np.float32)
    return out


# revision 14
# speedup vs baseline: 1.3077x; 1.3077x over previous
"""Bass/Trainium2 kernel for nn_BuildLstmUnrollNet.

Problem: 2-layer LSTM, unrolled T=11 steps with per-step (non-shared)
weights, B=8192, R=425, IN=20.  Output block t is the last-layer h
*before* step t, so only steps 0..9 need computing.

Strategy (data-parallel over batch, 8 cores x 1024 rows):
  - Step 0 runs in bf16: its matmul operands are the *initial* states,
    which are unbounded N(0,1) draws -- fp8 there costs ~2.5e-2 rel err
    (and the large initial |c| amplifies step-0 gate errors through the
    forget gate).  Steps 1..9 run the gate matmuls in fp8-e4m3
    DoubleRow (both operands fp8, 256-deep contraction per pass): after
    step 0 every h is a tanh*sigmoid product bounded by 1, and e4m3
    keeps the end-to-end rel err ~4e-3 (vs the 2e-2 gate).
  - States kept batch-major in ONE packed bf16 buffer per m-tile:
    cols [h0(425) | 1.0 | x(20) | h1(425) | 1.0 | pad] = 896 = 7*128.
    The second 1.0 (col 871) pairs with a host-precomputed fp8 residual
    weight row that cancels most of layer-1's bias quantization error.
  - Gates are computed batch-major in PSUM with transposed activations
    stationary (lhsT) and weights moving: layer 0 contracts packed rows
    0..511, layer 1 rows 0..1023 (chunk 7 is a zeroed pad chunk so
    layer 1 is exactly 4 DoubleRow passes).
  - The recurrent transpose h -> hT runs on the TENSOR ENGINE
    (transpose-mode matmuls against an identity, 128x128 blocks into a
    PSUM staging strip) and a vector-engine copy moves PSUM -> SBUF with
    the bf16 -> fp8 conversion fused (on the DVE: GPSIMD cannot access
    PSUM).  No DRAM bounce, no DMA-transpose
    queues: the per-m-tile chain is h-write (DVE) -> 3-4 PE transposes
    -> 1 Pool copy, all per-m pipelined.  (A DMA-transpose version lost
    ~18us/step to in-order DMA-queue stalls.)
  - PSUM budget (16 KiB/partition, bank-aligned): two buffers of
    [gates 1792 f32 | 512-col bf16 transpose strip] = 8 KiB each.
  - The whole unroll is ONE software-pipelined stream of (step, layer,
    m-tile) stages: stage s issues [PE transposes of stage s-3] [gate
    matmuls of s] [sig/tanh + c-update of s] [tanh(c) + h-write + DMAs
    of s-2], which keeps the Activation engine (the bottleneck after
    the fp8 matmuls) free of head-of-line stalls.
  - Cell math all-bf16 (2x DVE mode); output DMA reads the packed h1
    slice (out tensor is bf16, upconverted on the host).

kernel(**inputs) takes full-size numpy inputs, packs/shards on the
host, runs the same program SPMD on cores 0..7, and reassembles the
full [8192, 4675] fp32 output (block 0 comes from the initial state).
"""

import numpy as np
import ml_dtypes

BF16 = ml_dtypes.bfloat16
FP8 = ml_dtypes.float8_e4m3

B = 8192
NCORES = 8
BC = B // NCORES          # batch rows per core (1024)
NB = BC // 128            # m-tiles per core (8)
R = 425
IN = 20
GN = 4 * R                # 1700 gate columns
H1OFF = R + 1 + IN        # 446: h1 col offset in the packed state block
HC = 896                  # packed state block width (7*128)
NKC = 7                   # transposed K-chunks holding real data
NK0 = 4                   # layer-0 K-chunks (rows 0..511)
NKT = 11                  # step-0 bf16 weight K-blocks (4 + 7)
NW8 = 12                  # fp8 weight K-blocks (4 + 8, block 11 zero)
NSTEPS = 10
# N chunks of each 850-wide gate half ([i|f] then [o|g]), one PSUM
# bank each
NCHUNKS = [(0, 512), (512, 338)]

# set by test.py to profile; results stashed in LAST_RESULT
TRACE = False
LAST_RESULT = None


def build_bass(n_steps=NSTEPS, finalize=True):
    import concourse.bacc as bacc
    import concourse.mybir as mybir
    import concourse.tile as tile

    f32 = mybir.dt.float32
    bf16 = mybir.dt.bfloat16
    fp8 = mybir.dt.float8e4
    Sig = mybir.ActivationFunctionType.Sigmoid
    Tanh = mybir.ActivationFunctionType.Tanh
    DR = mybir.MatmulPerfMode.DoubleRow

    nc = bacc.Bacc()

    n8 = max(n_steps - 1, 1)
    w0_d = nc.declare_dram_parameter("w0", [128, NKT * GN], bf16, False)
    w8_d = nc.declare_dram_parameter("w8", [n8, 128, NW8 * GN], fp8, False)
    hci_d = nc.declare_dram_parameter("hci", [128, NB * HC], bf16, False)
    htci_d = nc.declare_dram_parameter("htci", [128, NKC * BC], bf16, False)
    c0i_d = nc.declare_dram_parameter("c0i", [128, NB * R], bf16, False)
    c1i_d = nc.declare_dram_parameter("c1i", [128, NB * R], bf16, False)
    eye_d = nc.declare_dram_parameter("eye", [128, 128], bf16, False)
    out_d = nc.declare_dram_parameter("out", [BC, n_steps * R], bf16, True)

    with tile.TileContext(nc) as tc:
        with (
            tc.tile_pool(name="consts", bufs=1) as consts,
            tc.tile_pool(name="wpool", bufs=2) as wpool,
            tc.tile_pool(name="gpsum", bufs=1, space="PSUM") as gpsum,
            tc.tile_pool(name="tmp", bufs=3) as tmp,
        ):
            # persistent state tiles
            hs_t = consts.tile([128, NB * HC], bf16)    # packed batch-major
            htc = consts.tile([128, NKC, BC], bf16)     # bf16 lhsT (step 0)
            htc8 = consts.tile([128, 8, BC], fp8)       # fp8 lhsT (DR)
            c0 = consts.tile([128, NB * R], bf16)
            c1 = consts.tile([128, NB * R], bf16)
            eye = consts.tile([128, 128], bf16)
            w0t = consts.tile([128, NKT * GN], bf16)    # step-0 weights

            # PSUM layout (8 banks, nothing shares a bank):
            # 3 rotating gate-half buffers (2 banks each: 850 f32 used)
            # + 2 transpose strips (1 bank each)
            gh0 = gpsum.tile([128, 1024], f32)
            gh1 = gpsum.tile([128, 1024], f32)
            gh2 = gpsum.tile([128, 1024], f32)
            tpA = gpsum.tile([128, 4, 128], bf16)
            tpB = gpsum.tile([128, 4, 128], bf16)
            ghbuf = [gh0, gh1, gh2]
            tpbuf = [tpA, tpB]

            # init DMAs on the SP (HWDGE) queue while the weight tables
            # stream on the Pool (SWDGE) queue
            for k in range(NKC):
                nc.sync.dma_start(htc[:, k, :], htci_d[:, k * BC:(k + 1) * BC])
            nc.sync.dma_start(c0[:], c0i_d[:])
            nc.sync.dma_start(hs_t[:], hci_d[:])
            nc.sync.dma_start(c1[:], c1i_d[:])
            nc.sync.dma_start(eye[:], eye_d[:])

            # step-0 weights, split per k-block so matmuls start early
            for k in range(NKT):
                nc.gpsimd.dma_start(w0t[:, k * GN:(k + 1) * GN],
                                    w0_d[:, k * GN:(k + 1) * GN])
            # first fp8 table (for step 1)
            w8tab = {}
            if n_steps > 1:
                w8cur = wpool.tile([128, NW8, GN], fp8, tag="w8")
                for c in range(4):
                    nc.gpsimd.dma_start(w8cur[:, 3 * c:3 * (c + 1), :],
                                        w8_d[0][:, 3 * c * GN:3 * (c + 1) * GN])
                w8tab[1] = w8cur
            # zero pad chunk for layer-1's 4th DoubleRow pass
            nc.gpsimd.memset(htc8[:, 7, :], 0.0)

            # PE warm-up while init DMAs land
            warm = consts.tile([128, 128], bf16)
            nc.vector.memset(warm[:], 0.0)
            for i in range(20):
                nc.tensor.matmul(gh0[:, 0:128], warm[:], warm[:],
                                 start=True, stop=True)

            def mm(gh, t, layer, m):
                """Gate matmuls into the two 850-col PSUM halves."""
                if t == 0:
                    if layer == 0:
                        kplan = [(k, k) for k in range(NK0)]
                    else:
                        kplan = ([(k, NK0 + k) for k in range(NK0, NKC)]
                                 + [(k, NK0 + k) for k in range(NK0)])
                    nk = len(kplan)
                    for ki, (kk, wk) in enumerate(kplan):
                        lhsT = htc[:, kk, m * 128:(m + 1) * 128]
                        for h in range(2):
                            for (no, nw) in NCHUNKS:
                                o = wk * GN + 850 * h + no
                                nc.tensor.matmul(
                                    gh[h][:, no:no + nw], lhsT,
                                    w0t[:, o:o + nw],
                                    start=(ki == 0), stop=(ki == nk - 1))
                else:
                    w8t = w8tab[t]
                    jplan = [0, 1] if layer == 0 else [2, 3, 0, 1]
                    nj = len(jplan)
                    for ji, j in enumerate(jplan):
                        wb = 2 * j + (0 if layer == 0 else 4)
                        lhsT = htc8[:, 2 * j:2 * j + 2, m * 128:(m + 1) * 128]
                        for h in range(2):
                            for (no, nw) in NCHUNKS:
                                o = 850 * h + no
                                nc.tensor.matmul(
                                    gh[h][:, no:no + nw], lhsT,
                                    w8t[:, wb:wb + 2, o:o + nw],
                                    start=(ji == 0), stop=(ji == nj - 1),
                                    perf_mode=DR)

            def cell_a(gh, layer, m):
                """Gate nonlinearities + c update (issued in-stage).
                half0 = [i|f], half1 = [o|g]."""
                cst = c0 if layer == 0 else c1
                cs = cst[:, m * R:(m + 1) * R]
                tsig = tmp.tile([128, 2 * R], bf16, tag="tsig")
                nc.scalar.activation(tsig[:], gh[0][:, 0:2 * R], Sig)
                tso = tmp.tile([128, R], bf16, tag="tso")
                nc.scalar.activation(tso[:], gh[1][:, 0:R], Sig)
                tg = tmp.tile([128, R], bf16, tag="tg")
                nc.scalar.activation(tg[:], gh[1][:, R:2 * R], Tanh)
                tfc = tmp.tile([128, R], bf16, tag="tfc")
                nc.vector.tensor_mul(tfc[:], tsig[:, R:2 * R], cs)
                tig = tmp.tile([128, R], bf16, tag="tig")
                nc.vector.tensor_mul(tig[:], tsig[:, 0:R], tg[:])
                nc.vector.tensor_add(cs, tfc[:], tig[:])
                return tso

            def cell_b(tso, tp, t, layer, m):
                """tanh(c) + h write + out store (issued 2 stages later)."""
                cst = c0 if layer == 0 else c1
                cs = cst[:, m * R:(m + 1) * R]
                ttc = tmp.tile([128, R], bf16, tag="ttc")
                nc.scalar.activation(ttc[:], cs, Tanh)
                off = m * HC + (0 if layer == 0 else H1OFF)
                nc.vector.tensor_mul(hs_t[:, off:off + R],
                                     tso[:], ttc[:])
                if layer == 1:
                    nc.sync.dma_start(
                        out_d[m * 128:(m + 1) * 128, t * R:(t + 1) * R],
                        hs_t[:, m * HC + H1OFF:m * HC + H1OFF + R])

            def cell_c(tp, t, layer, m):
                """PE-transpose of the freshly written packed-state block
                into the PSUM strip, then one Pool copy into the fp8 lhsT
                (+ a bf16 copy during step 0, whose layer 1 reads bf16).
                Issued 3 stages later so the PE never waits on the DVE."""
                if layer == 0:
                    kk, nk = 0, NK0          # packed cols 0..511
                else:
                    if t >= n_steps - 1:
                        return
                    kk, nk = NK0, NKC - NK0  # packed cols 512..895
                for k in range(nk):
                    nc.tensor.transpose(
                        tp[:, k, :],
                        hs_t[:, m * HC + (kk + k) * 128:
                             m * HC + (kk + k + 1) * 128],
                        eye[:])
                # GPSIMD can't read PSUM; the fp8-converting copy runs
                # on the DVE (1x rate: fp8 + PSUM disable the fast modes)
                nc.vector.tensor_copy(
                    htc8[:, kk:kk + nk, m * 128:(m + 1) * 128],
                    tp[:, 0:nk, :])
                if t == 0 and layer == 0:
                    # step 0's layer 1 contracts h0' in bf16
                    nc.vector.tensor_copy(
                        htc[:, kk:kk + nk, m * 128:(m + 1) * 128],
                        tp[:, 0:nk, :])

            # ---- the software-pipelined stage stream ----
            stages = [(t, layer, m)
                      for t in range(n_steps)
                      for layer in range(2)
                      for m in range(NB)]
            pend_b = []        # [(tsig, tp, t, layer, m)] awaiting cell_b
            pend_c = []        # [(tp, t, layer, m)] awaiting cell_c
            for s, (t, layer, m) in enumerate(stages):
                if layer == 0 and m == 0 and t >= 1 and t + 1 < n_steps:
                    w8n = wpool.tile([128, NW8, GN], fp8, tag="w8")
                    for c in range(4):
                        nc.gpsimd.dma_start(
                            w8n[:, 3 * c:3 * (c + 1), :],
                            w8_d[t][:, 3 * c * GN:3 * (c + 1) * GN])
                    w8tab[t + 1] = w8n

                if len(pend_c) > 2:
                    cell_c(*pend_c.pop(0))
                gh = (ghbuf[(2 * s) % 3], ghbuf[(2 * s + 1) % 3])
                tp = tpbuf[s % 2]
                mm(gh, t, layer, m)
                tso = cell_a(gh, layer, m)
                if len(pend_b) > 1:
                    cell_b(*pend_b.pop(0))
                pend_b.append((tso, tp, t, layer, m))
                pend_c.append((tp, t, layer, m))
            for args in pend_b:
                cell_b(*args)
            for args in pend_c:
                cell_c(*args)
    if finalize:
        nc.finalize()
    return nc


def _pack_pf(a):
    """[BC, C] -> [128, NB*C] with m-tile m at cols m*C."""
    c = a.shape[1]
    return np.ascontiguousarray(
        a.reshape(NB, 128, c).transpose(1, 0, 2).reshape(128, NB * c))


def _pack_kt(a):
    """[BC, HC] -> transposed [128, NKC*BC] with K-chunk k at cols k*BC."""
    return np.ascontiguousarray(
        a.T.reshape(NKC, 128, BC).transpose(1, 0, 2).reshape(128, NKC * BC))


def prep_inputs(x, init_states_input, W_i2h0, b_i2h0, W_h2h0, b_h2h0,
                W_i2h1, b_i2h1, W_h2h1, b_h2h1, n_steps=NSTEPS):
    """Host-side packing.  Returns (in_maps, h1_init_full)."""
    x = np.asarray(x, np.float32)
    init = np.asarray(init_states_input, np.float32)
    W_i2h0 = np.asarray(W_i2h0, np.float32)
    b_i2h0 = np.asarray(b_i2h0, np.float32)
    W_h2h0 = np.asarray(W_h2h0, np.float32)
    b_h2h0 = np.asarray(b_h2h0, np.float32)
    W_i2h1 = np.asarray(W_i2h1, np.float32)
    b_i2h1 = np.asarray(b_i2h1, np.float32)
    W_h2h1 = np.asarray(W_h2h1, np.float32)
    b_h2h1 = np.asarray(b_h2h1, np.float32)

    # step-0 bf16 weight table: K-major blocks, transposed to [K, 4R],
    # rows matching the packed state layout [h0 | 1 | x | h1 | 1 | pad]
    Wd0 = np.zeros((NKT * 128, GN), np.float32)
    Wd0[0:R] = W_h2h0[0].T
    Wd0[R] = b_i2h0[0] + b_h2h0[0]
    Wd0[R + 1:R + 1 + IN] = W_i2h0[0].T
    o = NK0 * 128
    Wd0[o:o + R] = W_i2h1[0].T
    Wd0[o + R] = b_i2h1[0] + b_h2h1[0]
    Wd0[o + H1OFF:o + H1OFF + R] = W_h2h1[0].T
    w0_dev = np.ascontiguousarray(
        Wd0.reshape(NKT, 128, GN).transpose(1, 0, 2)
        .reshape(128, NKT * GN)).astype(BF16)

    # fp8 tables for steps 1..n-1: blocks 0..3 layer 0 (512 rows),
    # blocks 4..11 layer 1 (1024 rows incl. zero pad + bias residual)
    n8 = max(n_steps - 1, 1)
    Wd8 = np.zeros((n8, NW8 * 128, GN), np.float32)
    for t in range(1, n_steps):
        d = Wd8[t - 1]
        d[0:R] = W_h2h0[t].T
        d[R] = b_i2h0[t] + b_h2h0[t]
        d[R + 1:R + 1 + IN] = W_i2h0[t].T
        o = NK0 * 128
        d[o:o + R] = W_i2h1[t].T
        b1 = b_i2h1[t] + b_h2h1[t]
        d[o + R] = b1
        d[o + H1OFF:o + H1OFF + R] = W_h2h1[t].T
        # residual row (pairs with the 1.0 at packed col 871): cancels
        # most of the fp8 quantization error of the layer-1 bias row
        d[o + H1OFF + R] = b1 - b1.astype(FP8).astype(np.float32)
    w8_dev = np.ascontiguousarray(
        Wd8.reshape(n8, NW8, 128, GN).transpose(0, 2, 1, 3)
        .reshape(n8, 128, NW8 * GN)).astype(FP8)

    init4 = init.reshape(B, 4, R)
    h0_full, c0_full = init4[:, 0], init4[:, 1]
    h1_full, c1_full = init4[:, 2], init4[:, 3]

    eye = np.eye(128, dtype=np.float32).astype(BF16)

    in_maps = []
    for c in range(NCORES):
        sl = slice(c * BC, (c + 1) * BC)
        hcp = np.zeros((BC, HC), np.float32)
        hcp[:, 0:R] = h0_full[sl]
        hcp[:, R] = 1.0
        hcp[:, R + 1:R + 1 + IN] = x[sl]
        hcp[:, H1OFF:H1OFF + R] = h1_full[sl]
        hcp[:, H1OFF + R] = 1.0
        hcp = hcp.astype(BF16)
        in_maps.append({
            "w0": w0_dev,
            "w8": w8_dev,
            "hci": _pack_pf(hcp),
            "htci": _pack_kt(hcp),
            "c0i": _pack_pf(np.ascontiguousarray(c0_full[sl])).astype(BF16),
            "c1i": _pack_pf(np.ascontiguousarray(c1_full[sl])).astype(BF16),
            "eye": eye,
        })
    return in_maps, h1_full


def kernel(x, init_states_input, W_i2h0, b_i2h0, W_h2h0, b_h2h0,
           W_i2h1, b_i2h1, W_h2h1, b_h2h1):
    global LAST_RESULT
    from concourse.bass_utils import run_bass_kernel_spmd

    in_maps, h1_full = prep_inputs(
        x, init_states_input, W_i2h0, b_i2h0, W_h2h0, b_h2h0,
        W_i2h1, b_i2h1, W_h2h1, b_h2h1)

    nc = build_bass(NSTEPS)
    res = run_bass_kernel_spmd(nc, in_maps, list(range(NCORES)), trace=TRACE)
    LAST_RESULT = res

    out = np.empty((B, (NSTEPS + 1) * R), np.float32)
    out[:, 0:R] = h1_full
    for c in range(NCORES):
        out[c * BC:(c + 1) * BC, R:] = res.results[c]["out"].astype(np.float32)
    return out


# revision 21
# speedup vs baseline: 1.4312x; 1.0944x over previous
"""Bass/Trainium2 kernel for nn_BuildLstmUnrollNet.

Problem: 2-layer LSTM, unrolled T=11 steps with per-step (non-shared)
weights, B=8192, R=425, IN=20.  Output block t is the last-layer h
*before* step t, so only steps 0..9 need computing.

Strategy (data-parallel over batch, 8 cores x 1024 rows):
  - Step 0 runs in bf16: its matmul operands are the *initial* states,
    which are unbounded N(0,1) draws -- fp8 there costs ~2.5e-2 rel err
    (and the large initial |c| amplifies step-0 gate errors through the
    forget gate).  Steps 1..9 run the gate matmuls in fp8-e4m3
    DoubleRow (both operands fp8, 256-deep contraction per pass): after
    step 0 every h is a tanh*sigmoid product bounded by 1, and e4m3
    keeps the end-to-end rel err ~4e-3 (vs the 2e-2 gate).
  - States kept batch-major in ONE packed bf16 buffer per m-tile:
    cols [h0(425) | 1.0 | x(20) | h1(425) | 1.0 | pad] = 896 = 7*128.
    The second 1.0 (col 871) pairs with a host-precomputed fp8 residual
    weight row that cancels most of layer-1's bias quantization error.
  - Gates are computed batch-major in PSUM with transposed activations
    stationary (lhsT) and weights moving: layer 0 contracts packed rows
    0..511, layer 1 rows 0..1023 (chunk 7 is a zeroed pad chunk so
    layer 1 is exactly 4 DoubleRow passes).
  - The recurrent transpose h -> hT runs on the TENSOR ENGINE
    (transpose-mode matmuls against an identity, 128x128 blocks into a
    PSUM staging strip) and a vector-engine copy moves PSUM -> SBUF with
    the bf16 -> fp8 conversion fused (on the DVE: GPSIMD cannot access
    PSUM).  No DRAM bounce, no DMA-transpose
    queues: the per-m-tile chain is h-write (DVE) -> 3-4 PE transposes
    -> 1 Pool copy, all per-m pipelined.  (A DMA-transpose version lost
    ~18us/step to in-order DMA-queue stalls.)
  - PSUM budget (16 KiB/partition, bank-aligned): two buffers of
    [gates 1792 f32 | 512-col bf16 transpose strip] = 8 KiB each.
  - The whole unroll is ONE software-pipelined stream of (step, layer,
    m-tile) stages: stage s issues [PE transposes of stage s-3] [gate
    matmuls of s] [sig/tanh + c-update of s] [tanh(c) + h-write + DMAs
    of s-2], which keeps the Activation engine (the bottleneck after
    the fp8 matmuls) free of head-of-line stalls.
  - Cell math all-bf16 (2x DVE mode); output DMA reads the packed h1
    slice (out tensor is bf16, upconverted on the host).

kernel(**inputs) takes full-size numpy inputs, packs/shards on the
host, runs the same program SPMD on cores 0..7, and reassembles the
full [8192, 4675] fp32 output (block 0 comes from the initial state).
"""

import numpy as np
import ml_dtypes

BF16 = ml_dtypes.bfloat16
FP8 = ml_dtypes.float8_e4m3

B = 8192
NCORES = 8
BC = B // NCORES          # batch rows per core (1024)
NB = BC // 128            # m-tiles per core (8)
R = 425
IN = 20
GN = 4 * R                # 1700 gate columns
H1OFF = R + 1 + IN        # 446: h1 col offset in the packed state block
HC = 896                  # packed state block width (7*128)
NKC = 7                   # transposed K-chunks holding real data
NK0 = 4                   # layer-0 K-chunks (rows 0..511)
NKT = 11                  # step-0 bf16 weight K-blocks (4 + 7)
NW8 = 12                  # fp8 weight K-blocks (4 + 8, block 11 zero)
NSTEPS = 10
# N chunks of each 850-wide gate half ([i|f] then [o|g]), one PSUM
# bank each
NCHUNKS = [(0, 512), (512, 338)]

# set by test.py to profile; results stashed in LAST_RESULT
TRACE = False
LAST_RESULT = None


def build_bass(n_steps=NSTEPS, finalize=True):
    import concourse.bacc as bacc
    import concourse.mybir as mybir
    import concourse.tile as tile

    f32 = mybir.dt.float32
    bf16 = mybir.dt.bfloat16
    fp8 = mybir.dt.float8e4
    Sig = mybir.ActivationFunctionType.Sigmoid
    Tanh = mybir.ActivationFunctionType.Tanh
    DR = mybir.MatmulPerfMode.DoubleRow

    nc = bacc.Bacc()

    n8 = max(n_steps - 1, 1)
    w0_d = nc.declare_dram_parameter("w0", [128, NKT * GN], bf16, False)
    w8_d = nc.declare_dram_parameter("w8", [n8, 128, NW8 * GN], fp8, False)
    hci_d = nc.declare_dram_parameter("hci", [128, NB * HC], bf16, False)
    htci_d = nc.declare_dram_parameter("htci", [128, NKC * BC], bf16, False)
    c0i_d = nc.declare_dram_parameter("c0i", [128, NB * R], bf16, False)
    c1i_d = nc.declare_dram_parameter("c1i", [128, NB * R], bf16, False)
    eye_d = nc.declare_dram_parameter("eye", [128, 128], bf16, False)
    out_d = nc.declare_dram_parameter("out", [BC, n_steps * R], bf16, True)

    with tile.TileContext(nc) as tc:
        with (
            tc.tile_pool(name="consts", bufs=1) as consts,
            tc.tile_pool(name="wpool", bufs=2) as wpool,
            tc.tile_pool(name="gpsum", bufs=1, space="PSUM") as gpsum,
            tc.tile_pool(name="tmp", bufs=5) as tmp,
        ):
            # persistent state tiles
            hs_t = consts.tile([128, NB * HC], bf16)    # packed batch-major
            htc = consts.tile([128, NKC, BC], bf16)     # bf16 lhsT (step 0)
            htc8 = consts.tile([128, 8, BC], fp8)       # fp8 lhsT (DR)
            c0 = consts.tile([128, NB * R], bf16)
            c1 = consts.tile([128, NB * R], bf16)
            eye = consts.tile([128, 128], bf16)
            w0t = consts.tile([128, NKT * GN], bf16)    # step-0 weights

            # PSUM layout (8 banks, nothing shares a bank):
            # 3 rotating gate-half buffers (2 banks each: 850 f32 used)
            # + 2 transpose strips (1 bank each)
            gh0 = gpsum.tile([128, 1024], f32)
            gh1 = gpsum.tile([128, 1024], f32)
            gh2 = gpsum.tile([128, 1024], f32)
            tpA = gpsum.tile([128, 4, 128], bf16)
            tpB = gpsum.tile([128, 4, 128], bf16)
            ghbuf = [gh0, gh1, gh2]
            tpbuf = [tpA, tpB]

            # init DMAs on the SP (HWDGE) queue while the weight tables
            # stream on the Pool (SWDGE) queue
            for k in range(NKC):
                nc.sync.dma_start(htc[:, k, :], htci_d[:, k * BC:(k + 1) * BC])
            nc.sync.dma_start(c0[:], c0i_d[:])
            nc.sync.dma_start(hs_t[:], hci_d[:])
            nc.sync.dma_start(c1[:], c1i_d[:])
            nc.sync.dma_start(eye[:], eye_d[:])

            # step-0 weights, split per k-block so matmuls start early
            for k in range(NKT):
                nc.gpsimd.dma_start(w0t[:, k * GN:(k + 1) * GN],
                                    w0_d[:, k * GN:(k + 1) * GN])
            # first fp8 table (for step 1)
            w8tab = {}
            if n_steps > 1:
                w8cur = wpool.tile([128, NW8, GN], fp8, tag="w8")
                for c in range(4):
                    nc.gpsimd.dma_start(w8cur[:, 3 * c:3 * (c + 1), :],
                                        w8_d[0][:, 3 * c * GN:3 * (c + 1) * GN])
                w8tab[1] = w8cur
            # zero pad chunk for layer-1's 4th DoubleRow pass
            nc.gpsimd.memset(htc8[:, 7, :], 0.0)

            # PE warm-up while init DMAs land
            warm = consts.tile([128, 128], bf16)
            nc.vector.memset(warm[:], 0.0)
            for i in range(20):
                nc.tensor.matmul(gh0[:, 0:128], warm[:], warm[:],
                                 start=True, stop=True)

            def mm(gh, t, layer, m):
                """Gate matmuls into the two 850-col PSUM halves."""
                if t == 0:
                    if layer == 0:
                        kplan = [(k, k) for k in range(NK0)]
                    else:
                        kplan = ([(k, NK0 + k) for k in range(NK0, NKC)]
                                 + [(k, NK0 + k) for k in range(NK0)])
                    nk = len(kplan)
                    for ki, (kk, wk) in enumerate(kplan):
                        lhsT = htc[:, kk, m * 128:(m + 1) * 128]
                        for h in range(2):
                            for (no, nw) in NCHUNKS:
                                o = wk * GN + 850 * h + no
                                nc.tensor.matmul(
                                    gh[h][:, no:no + nw], lhsT,
                                    w0t[:, o:o + nw],
                                    start=(ki == 0), stop=(ki == nk - 1))
                else:
                    w8t = w8tab[t]
                    jplan = [0, 1] if layer == 0 else [2, 3, 0, 1]
                    nj = len(jplan)
                    for ji, j in enumerate(jplan):
                        wb = 2 * j + (0 if layer == 0 else 4)
                        lhsT = htc8[:, 2 * j:2 * j + 2, m * 128:(m + 1) * 128]
                        for h in range(2):
                            for (no, nw) in NCHUNKS:
                                o = 850 * h + no
                                nc.tensor.matmul(
                                    gh[h][:, no:no + nw], lhsT,
                                    w8t[:, wb:wb + 2, o:o + nw],
                                    start=(ji == 0), stop=(ji == nj - 1),
                                    perf_mode=DR)

            def cell_a(gh, layer, m):
                """Gate nonlinearities + c update (issued in-stage).
                half0 = [i|f], half1 = [o|2g]: the g columns are scaled
                by 2 at weight-pack time so tanh(g) = 2*sigmoid(2g)-1
                and BOTH halves take a single 850-col sigmoid."""
                cst = c0 if layer == 0 else c1
                cs = cst[:, m * R:(m + 1) * R]
                tsig = tmp.tile([128, 2 * R], bf16, tag="tsig")
                nc.scalar.activation(tsig[:], gh[0][:, 0:2 * R], Sig)
                tog = tmp.tile([128, 2 * R], bf16, tag="tog")
                nc.scalar.activation(tog[:], gh[1][:, 0:2 * R], Sig)
                tfc = tmp.tile([128, R], bf16, tag="tfc")
                nc.vector.tensor_mul(tfc[:], tsig[:, R:2 * R], cs)
                # i*tanh(g) = 2*(sig_i*sig_2g) - sig_i
                tp2 = tmp.tile([128, R], bf16, tag="tp2")
                nc.vector.tensor_mul(tp2[:], tsig[:, 0:R], tog[:, R:2 * R])
                tq = tmp.tile([128, R], bf16, tag="tq")
                nc.vector.scalar_tensor_tensor(
                    tq[:], tp2[:], 2.0, tsig[:, 0:R],
                    mybir.AluOpType.mult, mybir.AluOpType.subtract)
                nc.vector.tensor_add(cs, tfc[:], tq[:])
                return tog

            def cell_b2(a, b):
                """tanh(c) + h write + out store for TWO adjacent stages
                (same step+layer, m and m+1): batched 2-row APs halve the
                per-instruction overhead on the ACT/DVE/DMA paths."""
                (tog_a, _, t, layer, m) = a
                (tog_b, _, _, _, m2) = b
                assert m2 == m + 1
                cst = c0 if layer == 0 else c1
                cs2 = cst[:, m * R:(m + 2) * R]
                ttc = tmp.tile([128, 2, R], bf16, tag="ttc")
                nc.scalar.activation(ttc[:], cs2, Tanh)
                off = m * HC + (0 if layer == 0 else H1OFF)
                nc.vector.tensor_mul(hs_t[:, off:off + R],
                                     tog_a[:, 0:R], ttc[:, 0, :])
                nc.vector.tensor_mul(hs_t[:, off + HC:off + HC + R],
                                     tog_b[:, 0:R], ttc[:, 1, :])
                if layer == 1:
                    for mm_ in (m, m + 1):
                        nc.sync.dma_start(
                            out_d[mm_ * 128:(mm_ + 1) * 128,
                                  t * R:(t + 1) * R],
                            hs_t[:, mm_ * HC + H1OFF:mm_ * HC + H1OFF + R])

            def cell_c(tp, t, layer, m):
                """PE-transpose of the freshly written packed-state block
                into the PSUM strip, then one Pool copy into the fp8 lhsT
                (+ a bf16 copy during step 0, whose layer 1 reads bf16).
                Issued 3 stages later so the PE never waits on the DVE."""
                if layer == 0:
                    kk, nk = 0, NK0          # packed cols 0..511
                else:
                    if t >= n_steps - 1:
                        return
                    kk, nk = NK0, NKC - NK0  # packed cols 512..895
                for k in range(nk):
                    nc.tensor.transpose(
                        tp[:, k, :],
                        hs_t[:, m * HC + (kk + k) * 128:
                             m * HC + (kk + k + 1) * 128],
                        eye[:])
                # GPSIMD can't read PSUM; the fp8-converting copy runs
                # on the DVE (1x rate: fp8 + PSUM disable the fast modes)
                nc.vector.tensor_copy(
                    htc8[:, kk:kk + nk, m * 128:(m + 1) * 128],
                    tp[:, 0:nk, :])
                if t == 0 and layer == 0:
                    # step 0's layer 1 contracts h0' in bf16
                    nc.vector.tensor_copy(
                        htc[:, kk:kk + nk, m * 128:(m + 1) * 128],
                        tp[:, 0:nk, :])

            # ---- the software-pipelined stage stream ----
            stages = [(t, layer, m)
                      for t in range(n_steps)
                      for layer in range(2)
                      for m in range(NB)]
            pend_b = []        # [(tsig, tp, t, layer, m)] awaiting cell_b
            pend_c = []        # [(tp, t, layer, m)] awaiting cell_c
            for s, (t, layer, m) in enumerate(stages):
                if layer == 0 and m == 0 and t >= 1 and t + 1 < n_steps:
                    w8n = wpool.tile([128, NW8, GN], fp8, tag="w8")
                    for c in range(4):
                        nc.gpsimd.dma_start(
                            w8n[:, 3 * c:3 * (c + 1), :],
                            w8_d[t][:, 3 * c * GN:3 * (c + 1) * GN])
                    w8tab[t + 1] = w8n

                if len(pend_c) > 4:
                    cell_c(*pend_c.pop(0))
                gh = (ghbuf[(2 * s) % 3], ghbuf[(2 * s + 1) % 3])
                tp = tpbuf[s % 2]
                mm(gh, t, layer, m)
                tog = cell_a(gh, layer, m)
                if len(pend_b) > 3:
                    a = pend_b.pop(0)
                    b = pend_b.pop(0)
                    cell_b2(a, b)
                pend_b.append((tog, tp, t, layer, m))
                pend_c.append((tp, t, layer, m))
            while pend_b:
                a = pend_b.pop(0)
                b = pend_b.pop(0)
                cell_b2(a, b)
            for args in pend_c:
                cell_c(*args)
    if finalize:
        nc.finalize()
    return nc


def _pack_pf(a):
    """[BC, C] -> [128, NB*C] with m-tile m at cols m*C."""
    c = a.shape[1]
    return np.ascontiguousarray(
        a.reshape(NB, 128, c).transpose(1, 0, 2).reshape(128, NB * c))


def _pack_kt(a):
    """[BC, HC] -> transposed [128, NKC*BC] with K-chunk k at cols k*BC."""
    return np.ascontiguousarray(
        a.T.reshape(NKC, 128, BC).transpose(1, 0, 2).reshape(128, NKC * BC))


def prep_inputs(x, init_states_input, W_i2h0, b_i2h0, W_h2h0, b_h2h0,
                W_i2h1, b_i2h1, W_h2h1, b_h2h1, n_steps=NSTEPS):
    """Host-side packing.  Returns (in_maps, h1_init_full)."""
    x = np.asarray(x, np.float32)
    init = np.asarray(init_states_input, np.float32)
    W_i2h0 = np.asarray(W_i2h0, np.float32)
    b_i2h0 = np.asarray(b_i2h0, np.float32)
    W_h2h0 = np.asarray(W_h2h0, np.float32)
    b_h2h0 = np.asarray(b_h2h0, np.float32)
    W_i2h1 = np.asarray(W_i2h1, np.float32)
    b_i2h1 = np.asarray(b_i2h1, np.float32)
    W_h2h1 = np.asarray(W_h2h1, np.float32)
    b_h2h1 = np.asarray(b_h2h1, np.float32)

    # step-0 bf16 weight table: K-major blocks, transposed to [K, 4R],
    # rows matching the packed state layout [h0 | 1 | x | h1 | 1 | pad]
    Wd0 = np.zeros((NKT * 128, GN), np.float32)
    Wd0[0:R] = W_h2h0[0].T
    Wd0[R] = b_i2h0[0] + b_h2h0[0]
    Wd0[R + 1:R + 1 + IN] = W_i2h0[0].T
    o = NK0 * 128
    Wd0[o:o + R] = W_i2h1[0].T
    Wd0[o + R] = b_i2h1[0] + b_h2h1[0]
    Wd0[o + H1OFF:o + H1OFF + R] = W_h2h1[0].T
    Wd0[:, 3 * R:] *= 2.0          # g columns: tanh via 2*sig(2x)-1
    w0_dev = np.ascontiguousarray(
        Wd0.reshape(NKT, 128, GN).transpose(1, 0, 2)
        .reshape(128, NKT * GN)).astype(BF16)

    # fp8 tables for steps 1..n-1: blocks 0..3 layer 0 (512 rows),
    # blocks 4..11 layer 1 (1024 rows incl. zero pad + bias residual)
    n8 = max(n_steps - 1, 1)
    Wd8 = np.zeros((n8, NW8 * 128, GN), np.float32)
    for t in range(1, n_steps):
        d = Wd8[t - 1]
        d[0:R] = W_h2h0[t].T
        d[R] = b_i2h0[t] + b_h2h0[t]
        d[R + 1:R + 1 + IN] = W_i2h0[t].T
        o = NK0 * 128
        d[o:o + R] = W_i2h1[t].T
        b1 = b_i2h1[t] + b_h2h1[t]
        d[o + R] = b1
        d[o + H1OFF:o + H1OFF + R] = W_h2h1[t].T
        # residual row (pairs with the 1.0 at packed col 871): cancels
        # most of the fp8 quantization error of the layer-1 bias row
        d[o + H1OFF + R] = b1 - b1.astype(FP8).astype(np.float32)
    Wd8[:, :, 3 * R:] *= 2.0       # g columns: tanh via 2*sig(2x)-1
    w8_dev = np.ascontiguousarray(
        Wd8.reshape(n8, NW8, 128, GN).transpose(0, 2, 1, 3)
        .reshape(n8, 128, NW8 * GN)).astype(FP8)

    init4 = init.reshape(B, 4, R)
    h0_full, c0_full = init4[:, 0], init4[:, 1]
    h1_full, c1_full = init4[:, 2], init4[:, 3]

    eye = np.eye(128, dtype=np.float32).astype(BF16)

    in_maps = []
    for c in range(NCORES):
        sl = slice(c * BC, (c + 1) * BC)
        hcp = np.zeros((BC, HC), np.float32)
        hcp[:, 0:R] = h0_full[sl]
        hcp[:, R] = 1.0
        hcp[:, R + 1:R + 1 + IN] = x[sl]
        hcp[:, H1OFF:H1OFF + R] = h1_full[sl]
        hcp[:, H1OFF + R] = 1.0
        hcp = hcp.astype(BF16)
        in_maps.append({
            "w0": w0_dev,
            "w8": w8_dev,
            "hci": _pack_pf(hcp),
            "htci": _pack_kt(hcp),
            "c0i": _pack_pf(np.ascontiguousarray(c0_full[sl])).astype(BF16),
            "c1i": _pack_pf(np.ascontiguousarray(c1_full[sl])).astype(BF16),
            "eye": eye,
        })
    return in_maps, h1_full


def kernel(x, init_states_input, W_i2h0, b_i2h0, W_h2h0, b_h2h0,
           W_i2h1, b_i2h1, W_h2h1, b_h2h1):
    global LAST_RESULT
    from concourse.bass_utils import run_bass_kernel_spmd

    in_maps, h1_full = prep_inputs(
        x, init_states_input, W_i2h0, b_i2h0, W_h2h0, b_h2h0,
        W_i2h1, b_i2h1, W_h2h1, b_h2h1)

    nc = build_bass(NSTEPS)
    res = run_bass_kernel_spmd(nc, in_maps, list(range(NCORES)), trace=TRACE)
    LAST_RESULT = res

    out = np.empty((B, (NSTEPS + 1) * R), np.float32)
    out[:, 0:R] = h1_full
    for c in range(NCORES):
        out[c * BC:(c + 1) * BC, R:] = res.results[c]["out"].astype(np.float32)
    return out


# revision 29
# speedup vs baseline: 1.4419x; 1.0075x over previous
"""Bass/Trainium2 kernel for nn_BuildLstmUnrollNet.

Problem: 2-layer LSTM, unrolled T=11 steps with per-step (non-shared)
weights, B=8192, R=425, IN=20.  Output block t is the last-layer h
*before* step t, so only steps 0..9 need computing.

Strategy (data-parallel over batch, 8 cores x 1024 rows):
  - Step 0 runs in bf16: its matmul operands are the *initial* states,
    which are unbounded N(0,1) draws -- fp8 there costs ~2.5e-2 rel err
    (and the large initial |c| amplifies step-0 gate errors through the
    forget gate).  Steps 1..9 run the gate matmuls in fp8-e4m3
    DoubleRow (both operands fp8, 256-deep contraction per pass): after
    step 0 every h is a tanh*sigmoid product bounded by 1, and e4m3
    keeps the end-to-end rel err ~4e-3 (vs the 2e-2 gate).
  - States kept batch-major in ONE packed bf16 buffer per m-tile:
    cols [h0(425) | 1.0 | x(20) | h1(425) | 1.0 | pad] = 896 = 7*128.
    The second 1.0 (col 871) pairs with a host-precomputed fp8 residual
    weight row that cancels most of layer-1's bias quantization error.
  - Gates are computed batch-major in PSUM with transposed activations
    stationary (lhsT) and weights moving: layer 0 contracts packed rows
    0..511, layer 1 rows 0..1023 (chunk 7 is a zeroed pad chunk so
    layer 1 is exactly 4 DoubleRow passes).
  - The recurrent transpose h -> hT runs on the TENSOR ENGINE
    (transpose-mode matmuls against an identity, 128x128 blocks into a
    PSUM staging strip) and a vector-engine copy moves PSUM -> SBUF with
    the bf16 -> fp8 conversion fused (on the DVE: GPSIMD cannot access
    PSUM).  No DRAM bounce, no DMA-transpose
    queues: the per-m-tile chain is h-write (DVE) -> 3-4 PE transposes
    -> 1 Pool copy, all per-m pipelined.  (A DMA-transpose version lost
    ~18us/step to in-order DMA-queue stalls.)
  - PSUM budget (16 KiB/partition, bank-aligned): two buffers of
    [gates 1792 f32 | 512-col bf16 transpose strip] = 8 KiB each.
  - The whole unroll is ONE software-pipelined stream of (step, layer,
    m-tile) stages: stage s issues [PE transposes of stage s-3] [gate
    matmuls of s] [sig/tanh + c-update of s] [tanh(c) + h-write + DMAs
    of s-2], which keeps the Activation engine (the bottleneck after
    the fp8 matmuls) free of head-of-line stalls.
  - Cell math all-bf16 (2x DVE mode); output DMA reads the packed h1
    slice (out tensor is bf16, upconverted on the host).

kernel(**inputs) takes full-size numpy inputs, packs/shards on the
host, runs the same program SPMD on cores 0..7, and reassembles the
full [8192, 4675] fp32 output (block 0 comes from the initial state).
"""

import numpy as np
import ml_dtypes

BF16 = ml_dtypes.bfloat16
FP8 = ml_dtypes.float8_e4m3

B = 8192
NCORES = 8
BC = B // NCORES          # batch rows per core (1024)
NB = BC // 128            # m-tiles per core (8)
R = 425
IN = 20
GN = 4 * R                # 1700 gate columns
H1OFF = R + 1 + IN        # 446: h1 col offset in the packed state block
HC = 896                  # packed state block width (7*128)
NKC = 7                   # transposed K-chunks holding real data
NK0 = 4                   # layer-0 K-chunks (rows 0..511)
NKT = 11                  # step-0 bf16 weight K-blocks (4 + 7)
NW8 = 12                  # fp8 weight K-blocks (4 + 8, block 11 zero)
NSTEPS = 10
# N chunks of each 850-wide gate half ([i|f] then [o|g]), one PSUM
# bank each
NCHUNKS = [(0, 512), (512, 338)]

# set by test.py to profile; results stashed in LAST_RESULT
TRACE = False
LAST_RESULT = None


def build_bass(n_steps=NSTEPS, finalize=True):
    import concourse.bacc as bacc
    import concourse.mybir as mybir
    import concourse.tile as tile

    f32 = mybir.dt.float32
    bf16 = mybir.dt.bfloat16
    fp8 = mybir.dt.float8e4
    Sig = mybir.ActivationFunctionType.Sigmoid
    Tanh = mybir.ActivationFunctionType.Tanh
    DR = mybir.MatmulPerfMode.DoubleRow

    nc = bacc.Bacc()

    n8 = max(n_steps - 1, 1)
    w0_d = nc.declare_dram_parameter("w0", [128, 7 * GN], bf16, False)
    w8_d = nc.declare_dram_parameter("w8", [n8, 128, NW8 * GN], fp8, False)
    w80_d = nc.declare_dram_parameter("w80", [128, 8 * GN], fp8, False)
    a0_d = nc.declare_dram_parameter("a0", [128, 8 * BC], fp8, False)
    htci_d = nc.declare_dram_parameter("htci", [128, 3 * BC], bf16, False)
    hci_d = nc.declare_dram_parameter("hci", [128, NB * HC], bf16, False)
    c0i_d = nc.declare_dram_parameter("c0i", [128, NB * R], bf16, False)
    c1i_d = nc.declare_dram_parameter("c1i", [128, NB * R], bf16, False)
    eye_d = nc.declare_dram_parameter("eye", [128, 128], bf16, False)
    out_d = nc.declare_dram_parameter("out", [BC, n_steps * R], bf16, True)

    with tile.TileContext(nc) as tc:
        with (
            tc.tile_pool(name="consts", bufs=1) as consts,
            tc.tile_pool(name="wpool", bufs=2) as wpool,
            tc.tile_pool(name="gpsum", bufs=1, space="PSUM") as gpsum,
            tc.tile_pool(name="tmp", bufs=5) as tmp,
        ):
            # persistent state tiles
            hs_t = consts.tile([128, NB * HC], bf16)    # packed batch-major
            htc = consts.tile([128, NKC, BC], bf16)     # bf16 lhsT (t=0 L1)
            htc8 = consts.tile([128, 8, BC], fp8)       # fp8 lhsT (DR)
            c0 = consts.tile([128, NB * R], bf16)
            c1 = consts.tile([128, NB * R], bf16)
            eye = consts.tile([128, 128], bf16)
            w0t = consts.tile([128, 7, GN], bf16)       # t=0 L1 weights
            # step-0 layer-0 fp8 operands: host-packed initial state (+
            # exact 1/8-scaled residual chunks) and its weight blocks
            a0t = consts.tile([128, 8, BC], fp8)
            w80t = consts.tile([128, 8, GN], fp8)

            # PSUM layout (8 banks, nothing shares a bank):
            # 3 rotating gate-half buffers (2 banks each: 850 f32 used)
            # + 2 transpose strips (1 bank each)
            gh0 = gpsum.tile([128, 1024], f32)
            gh1 = gpsum.tile([128, 1024], f32)
            gh2 = gpsum.tile([128, 1024], f32)
            tpA = gpsum.tile([128, 4, 128], bf16)
            tpB = gpsum.tile([128, 4, 128], bf16)
            ghbuf = [gh0, gh1, gh2]
            tpbuf = [tpA, tpB]

            # init DMAs on the SP (HWDGE) queue, quarter-interleaved in
            # conveyor order (stage s consumes c0/hs_t slice s at ~2.3us
            # per stage), while weights stream on the Pool (SWDGE) queue
            nc.sync.dma_start(a0t[:, 0:2, :], a0_d[:, 0:2 * BC])
            nc.sync.dma_start(eye[:], eye_d[:])
            nc.sync.dma_start(c0[:], c0i_d[:])
            for k in range(1, 4):
                nc.sync.dma_start(a0t[:, 2 * k:2 * k + 2, :],
                                  a0_d[:, 2 * k * BC:(2 * k + 2) * BC])
            nc.sync.dma_start(hs_t[:, 0:4 * HC], hci_d[:, 0:4 * HC])
            nc.sync.dma_start(hs_t[:, 4 * HC:], hci_d[:, 4 * HC:])
            for k in range(3):
                nc.sync.dma_start(htc[:, NK0 + k, :],
                                  htci_d[:, k * BC:(k + 1) * BC])
            nc.sync.dma_start(c1[:], c1i_d[:])

            # step-0 weights, split so the first matmuls start early
            for k in range(4):
                nc.gpsimd.dma_start(w80t[:, 2 * k:2 * k + 2, :],
                                    w80_d[:, 2 * k * GN:(2 * k + 2) * GN])
            nc.gpsimd.dma_start(w0t[:], w0_d[:])
            # first fp8 table (for step 1)
            w8tab = {}
            if n_steps > 1:
                w8cur = wpool.tile([128, NW8, GN], fp8, tag="w8")
                for c in range(4):
                    nc.gpsimd.dma_start(w8cur[:, 3 * c:3 * (c + 1), :],
                                        w8_d[0][:, 3 * c * GN:3 * (c + 1) * GN])
                w8tab[1] = w8cur
            # zero pad chunk for layer-1's 4th DoubleRow pass
            nc.gpsimd.memset(htc8[:, 7, :], 0.0)

            # PE warm-up while init DMAs land
            warm = consts.tile([128, 128], bf16)
            nc.vector.memset(warm[:], 0.0)
            for i in range(20):
                nc.tensor.matmul(gh0[:, 0:128], warm[:], warm[:],
                                 start=True, stop=True)

            def mm(gh, t, layer, m):
                """Gate matmuls into the two 850-col PSUM halves."""
                if t == 0:
                    if layer == 0:
                        # 4 DR passes on the host-packed fp8 initial state:
                        # main chunks (0,1),(2,3) then exact-residual
                        # chunks (4,5),(6,7)
                        for ji in range(4):
                            lhsT = a0t[:, 2 * ji:2 * ji + 2,
                                       m * 128:(m + 1) * 128]
                            for h in range(2):
                                for (no, nw) in NCHUNKS:
                                    o = 850 * h + no
                                    nc.tensor.matmul(
                                        gh[h][:, no:no + nw], lhsT,
                                        w80t[:, 2 * ji:2 * ji + 2, o:o + nw],
                                        start=(ji == 0), stop=(ji == 3),
                                        perf_mode=DR)
                    else:
                        # h1 part first (3 bf16 passes on the host-init
                        # transposed state: unbounded h1 + its weights
                        # stay bf16), then the h0' part (4 bf16 passes on
                        # the mid-step transposed device state)
                        kplan = ([(NK0 + k, NK0 + k) for k in range(3)]
                                 + [(k, k) for k in range(NK0)])
                        for ki, (kk, wk) in enumerate(kplan):
                            lhsT = htc[:, kk, m * 128:(m + 1) * 128]
                            for h in range(2):
                                for (no, nw) in NCHUNKS:
                                    o = 850 * h + no
                                    nc.tensor.matmul(
                                        gh[h][:, no:no + nw], lhsT,
                                        w0t[:, wk, o:o + nw],
                                        start=(ki == 0), stop=(ki == 6))
                else:
                    w8t = w8tab[t]
                    jplan = [0, 1] if layer == 0 else [2, 3, 0, 1]
                    nj = len(jplan)
                    for ji, j in enumerate(jplan):
                        wb = 2 * j + (0 if layer == 0 else 4)
                        lhsT = htc8[:, 2 * j:2 * j + 2, m * 128:(m + 1) * 128]
                        for h in range(2):
                            for (no, nw) in NCHUNKS:
                                o = 850 * h + no
                                nc.tensor.matmul(
                                    gh[h][:, no:no + nw], lhsT,
                                    w8t[:, wb:wb + 2, o:o + nw],
                                    start=(ji == 0), stop=(ji == nj - 1),
                                    perf_mode=DR)

            def cell_a(gh, layer, m):
                """Gate nonlinearities + c update (issued in-stage).
                half0 = [i|f], half1 = [o|2g]: the g columns are scaled
                by 2 at weight-pack time so tanh(g) = 2*sigmoid(2g)-1
                and BOTH halves take a single 850-col sigmoid."""
                cst = c0 if layer == 0 else c1
                cs = cst[:, m * R:(m + 1) * R]
                tsig = tmp.tile([128, 2 * R], bf16, tag="tsig")
                nc.scalar.activation(tsig[:], gh[0][:, 0:2 * R], Sig)
                tog = tmp.tile([128, 2 * R], bf16, tag="tog")
                nc.scalar.activation(tog[:], gh[1][:, 0:2 * R], Sig)
                tfc = tmp.tile([128, R], bf16, tag="tfc")
                nc.vector.tensor_mul(tfc[:], tsig[:, R:2 * R], cs)
                # i*tanh(g) = 2*(sig_i*sig_2g) - sig_i
                tp2 = tmp.tile([128, R], bf16, tag="tp2")
                nc.vector.tensor_mul(tp2[:], tsig[:, 0:R], tog[:, R:2 * R])
                tq = tmp.tile([128, R], bf16, tag="tq")
                nc.vector.scalar_tensor_tensor(
                    tq[:], tp2[:], 2.0, tsig[:, 0:R],
                    mybir.AluOpType.mult, mybir.AluOpType.subtract)
                nc.vector.tensor_add(cs, tfc[:], tq[:])
                return tog

            def cell_b2(a, b):
                """tanh(c) + h write + out store for TWO adjacent stages
                (same step+layer, m and m+1): batched 2-row APs halve the
                per-instruction overhead on the ACT/DVE/DMA paths."""
                (tog_a, _, t, layer, m) = a
                (tog_b, _, _, _, m2) = b
                assert m2 == m + 1
                cst = c0 if layer == 0 else c1
                cs2 = cst[:, m * R:(m + 2) * R]
                ttc = tmp.tile([128, 2, R], bf16, tag="ttc")
                nc.scalar.activation(ttc[:], cs2, Tanh)
                off = m * HC + (0 if layer == 0 else H1OFF)
                nc.vector.tensor_mul(hs_t[:, off:off + R],
                                     tog_a[:, 0:R], ttc[:, 0, :])
                nc.vector.tensor_mul(hs_t[:, off + HC:off + HC + R],
                                     tog_b[:, 0:R], ttc[:, 1, :])
                if layer == 1:
                    for mm_ in (m, m + 1):
                        nc.sync.dma_start(
                            out_d[mm_ * 128:(mm_ + 1) * 128,
                                  t * R:(t + 1) * R],
                            hs_t[:, mm_ * HC + H1OFF:mm_ * HC + H1OFF + R])

            def cell_c(tp, t, layer, m):
                """PE-transpose of the freshly written packed-state block
                into the PSUM strip, then one Pool copy into the fp8 lhsT
                (+ a bf16 copy during step 0, whose layer 1 reads bf16).
                Issued 3 stages later so the PE never waits on the DVE."""
                if layer == 0:
                    kk, nk = 0, NK0          # packed cols 0..511
                else:
                    if t >= n_steps - 1:
                        return
                    kk, nk = NK0, NKC - NK0  # packed cols 512..895
                for k in range(nk):
                    nc.tensor.transpose(
                        tp[:, k, :],
                        hs_t[:, m * HC + (kk + k) * 128:
                             m * HC + (kk + k + 1) * 128],
                        eye[:])
                # GPSIMD can't read PSUM; the fp8-converting copy runs
                # on the DVE (1x rate: fp8 + PSUM disable the fast modes)
                nc.vector.tensor_copy(
                    htc8[:, kk:kk + nk, m * 128:(m + 1) * 128],
                    tp[:, 0:nk, :])
                if t == 0 and layer == 0:
                    # step 0's layer 1 contracts h0' in bf16
                    nc.vector.tensor_copy(
                        htc[:, 0:NK0, m * 128:(m + 1) * 128],
                        tp[:, 0:NK0, :])

            # ---- the software-pipelined stage stream ----
            stages = [(t, layer, m)
                      for t in range(n_steps)
                      for layer in range(2)
                      for m in range(NB)]
            pend_b = []        # [(tsig, tp, t, layer, m)] awaiting cell_b
            pend_c = []        # [(tp, t, layer, m)] awaiting cell_c
            for s, (t, layer, m) in enumerate(stages):
                if layer == 0 and m == 0 and t >= 1 and t + 1 < n_steps:
                    w8n = wpool.tile([128, NW8, GN], fp8, tag="w8")
                    for c in range(4):
                        nc.gpsimd.dma_start(
                            w8n[:, 3 * c:3 * (c + 1), :],
                            w8_d[t][:, 3 * c * GN:3 * (c + 1) * GN])
                    w8tab[t + 1] = w8n

                if len(pend_c) > 4:
                    cell_c(*pend_c.pop(0))
                gh = (ghbuf[(2 * s) % 3], ghbuf[(2 * s + 1) % 3])
                tp = tpbuf[s % 2]
                mm(gh, t, layer, m)
                tog = cell_a(gh, layer, m)
                if len(pend_b) > 3:
                    a = pend_b.pop(0)
                    b = pend_b.pop(0)
                    cell_b2(a, b)
                pend_b.append((tog, tp, t, layer, m))
                pend_c.append((tp, t, layer, m))
            while pend_b:
                a = pend_b.pop(0)
                b = pend_b.pop(0)
                cell_b2(a, b)
            for args in pend_c:
                cell_c(*args)
    if finalize:
        nc.finalize()
    return nc


def _pack_pf(a):
    """[BC, C] -> [128, NB*C] with m-tile m at cols m*C."""
    c = a.shape[1]
    return np.ascontiguousarray(
        a.reshape(NB, 128, c).transpose(1, 0, 2).reshape(128, NB * c))


def _pack_kt(a):
    """[BC, HC] -> transposed [128, NKC*BC] with K-chunk k at cols k*BC."""
    return np.ascontiguousarray(
        a.T.reshape(NKC, 128, BC).transpose(1, 0, 2).reshape(128, NKC * BC))


def prep_inputs(x, init_states_input, W_i2h0, b_i2h0, W_h2h0, b_h2h0,
                W_i2h1, b_i2h1, W_h2h1, b_h2h1, n_steps=NSTEPS):
    """Host-side packing.  Returns (in_maps, h1_init_full)."""
    x = np.asarray(x, np.float32)
    init = np.asarray(init_states_input, np.float32)
    W_i2h0 = np.asarray(W_i2h0, np.float32)
    b_i2h0 = np.asarray(b_i2h0, np.float32)
    W_h2h0 = np.asarray(W_h2h0, np.float32)
    b_h2h0 = np.asarray(b_h2h0, np.float32)
    W_i2h1 = np.asarray(W_i2h1, np.float32)
    b_i2h1 = np.asarray(b_i2h1, np.float32)
    W_h2h1 = np.asarray(W_h2h1, np.float32)
    b_h2h1 = np.asarray(b_h2h1, np.float32)

    def q8f(a):
        return a.astype(FP8).astype(np.float32)

    # ---- step-0 weight tables ----
    # L0 rows 0..511: [W_h2h0[0].T | b0 | W_i2h0[0].T | 0], g cols x2
    Wl0 = np.zeros((512, GN), np.float32)
    Wl0[0:R] = W_h2h0[0].T
    b0s = b_i2h0[0] + b_h2h0[0]
    Wl0[R] = b0s
    Wl0[R + 1:R + 1 + IN] = W_i2h0[0].T
    Wl0[:, 3 * R:] *= 2.0
    # L1 rows 0..511 (bf16: h0' part + bias + h1 features 0..65)
    Wl1a = np.zeros((512, GN), np.float32)
    Wl1a[0:R] = W_i2h1[0].T
    Wl1a[R] = b_i2h1[0] + b_h2h1[0]
    Wl1a[H1OFF:512] = W_h2h1[0].T[0:512 - H1OFF]
    Wl1a[:, 3 * R:] *= 2.0
    # L1 h1 part rows (packed cols 512..895): W_h2h1[0].T[66..424] + 0
    Wl1h = np.zeros((384, GN), np.float32)
    Wl1h[0:R - (512 - H1OFF)] = W_h2h1[0].T[512 - H1OFF:]
    Wl1h[:, 3 * R:] *= 2.0
    w0_dev = np.ascontiguousarray(
        np.concatenate([Wl1a, Wl1h], axis=0)
        .reshape(7, 128, GN).transpose(1, 0, 2)
        .reshape(128, 7 * GN)).astype(BF16)

    # w80: blocks 0..3 = fp8 main L0, 4..7 = 1/8 residual weights
    W80 = np.zeros((8, 128, GN), np.float32)
    q = q8f(Wl0)
    W80[0:4] = q.reshape(4, 128, GN)
    W80[4:8] = (q / 8).reshape(4, 128, GN)
    W80[7][425 - 3 * 128] = Wl0[R] - q8f(Wl0[R])   # bias residual row
    w80_dev = np.ascontiguousarray(
        W80.transpose(1, 0, 2).reshape(128, 8 * GN)).astype(FP8)

    # ---- fp8 tables for steps 1..n-1 ----
    n8 = max(n_steps - 1, 1)
    Wd8 = np.zeros((n8, NW8 * 128, GN), np.float32)
    for t in range(1, n_steps):
        d = Wd8[t - 1]
        d[0:R] = W_h2h0[t].T
        d[R] = b_i2h0[t] + b_h2h0[t]
        d[R + 1:R + 1 + IN] = W_i2h0[t].T
        o = NK0 * 128
        d[o:o + R] = W_i2h1[t].T
        b1 = b_i2h1[t] + b_h2h1[t]
        d[o + R] = b1
        d[o + H1OFF:o + H1OFF + R] = W_h2h1[t].T
        # residual row (pairs with the 1.0 at packed col 871): cancels
        # most of the fp8 quantization error of the layer-1 bias row
        d[o + H1OFF + R] = b1 - b1.astype(FP8).astype(np.float32)
    Wd8[:, :, 3 * R:] *= 2.0       # g columns: tanh via 2*sig(2x)-1
    w8_dev = np.ascontiguousarray(
        Wd8.reshape(n8, NW8, 128, GN).transpose(0, 2, 1, 3)
        .reshape(n8, 128, NW8 * GN)).astype(FP8)

    init4 = init.reshape(B, 4, R)
    h0_full, c0_full = init4[:, 0], init4[:, 1]
    h1_full, c1_full = init4[:, 2], init4[:, 3]

    eye = np.eye(128, dtype=np.float32).astype(BF16)

    def kmaj8(block, resid_one_col=None):
        """[BC, 512] fp32 -> fp8 [128, 8*BC]: 4 K-major main chunks then
        4 residual chunks (8x the fp8 quantization error)."""
        q = block.astype(FP8).astype(np.float32)
        r8 = 8.0 * (block - q)
        if resid_one_col is not None:
            r8[:, resid_one_col] = 1.0
        both = np.concatenate(
            [q.T.reshape(4, 128, BC), r8.T.reshape(4, 128, BC)], axis=0)
        return np.ascontiguousarray(
            both.transpose(1, 0, 2).reshape(128, 8 * BC)).astype(FP8)

    in_maps = []
    for c in range(NCORES):
        sl = slice(c * BC, (c + 1) * BC)
        hcp = np.zeros((BC, HC), np.float32)
        hcp[:, 0:R] = h0_full[sl]
        hcp[:, R] = 1.0
        hcp[:, R + 1:R + 1 + IN] = x[sl]
        hcp[:, H1OFF:H1OFF + R] = h1_full[sl]
        hcp[:, H1OFF + R] = 1.0
        hcp = hcp.astype(BF16).astype(np.float32)
        in_maps.append({
            "w0": w0_dev,
            "w8": w8_dev,
            "w80": w80_dev,
            "a0": kmaj8(hcp[:, 0:512], resid_one_col=R),
            "htci": np.ascontiguousarray(
                hcp[:, 512:HC].T.reshape(3, 128, BC).transpose(1, 0, 2)
                .reshape(128, 3 * BC)).astype(BF16),
            "hci": _pack_pf(hcp.astype(BF16)),
            "c0i": _pack_pf(np.ascontiguousarray(c0_full[sl])).astype(BF16),
            "c1i": _pack_pf(np.ascontiguousarray(c1_full[sl])).astype(BF16),
            "eye": eye,
        })
    return in_maps, h1_full


def kernel(x, init_states_input, W_i2h0, b_i2h0, W_h2h0, b_h2h0,
           W_i2h1, b_i2h1, W_h2h1, b_h2h1):
    global LAST_RESULT
    from concourse.bass_utils import run_bass_kernel_spmd

    in_maps, h1_full = prep_inputs(
        x, init_states_input, W_i2h0, b_i2h0, W_h2h0, b_h2h0,
        W_i2h1, b_i2h1, W_h2h1, b_h2h1)

    nc = build_bass(NSTEPS)
    res = run_bass_kernel_spmd(nc, in_maps, list(range(NCORES)), trace=TRACE)
    LAST_RESULT = res

    out = np.empty((B, (NSTEPS + 1) * R), np.float32)
    out[:, 0:R] = h1_full
    for c in range(NCORES):
        out[c * BC:(c + 1) * BC, R:] = res.results[c]["out"].astype(np.float32)
    return out


# revision 32
# speedup vs baseline: 1.4447x; 1.0020x over previous
"""Bass/Trainium2 kernel for nn_BuildLstmUnrollNet.

Problem: 2-layer LSTM, unrolled T=11 steps with per-step (non-shared)
weights, B=8192, R=425, IN=20.  Output block t is the last-layer h
*before* step t, so only steps 0..9 need computing.

Strategy (data-parallel over batch, 8 cores x 1024 rows):
  - Step 0 runs in bf16: its matmul operands are the *initial* states,
    which are unbounded N(0,1) draws -- fp8 there costs ~2.5e-2 rel err
    (and the large initial |c| amplifies step-0 gate errors through the
    forget gate).  Steps 1..9 run the gate matmuls in fp8-e4m3
    DoubleRow (both operands fp8, 256-deep contraction per pass): after
    step 0 every h is a tanh*sigmoid product bounded by 1, and e4m3
    keeps the end-to-end rel err ~4e-3 (vs the 2e-2 gate).
  - States kept batch-major in ONE packed bf16 buffer per m-tile:
    cols [h0(425) | 1.0 | x(20) | h1(425) | 1.0 | pad] = 896 = 7*128.
    The second 1.0 (col 871) pairs with a host-precomputed fp8 residual
    weight row that cancels most of layer-1's bias quantization error.
  - Gates are computed batch-major in PSUM with transposed activations
    stationary (lhsT) and weights moving: layer 0 contracts packed rows
    0..511, layer 1 rows 0..1023 (chunk 7 is a zeroed pad chunk so
    layer 1 is exactly 4 DoubleRow passes).
  - The recurrent transpose h -> hT runs on the TENSOR ENGINE
    (transpose-mode matmuls against an identity, 128x128 blocks into a
    PSUM staging strip) and a vector-engine copy moves PSUM -> SBUF with
    the bf16 -> fp8 conversion fused (on the DVE: GPSIMD cannot access
    PSUM).  No DRAM bounce, no DMA-transpose
    queues: the per-m-tile chain is h-write (DVE) -> 3-4 PE transposes
    -> 1 Pool copy, all per-m pipelined.  (A DMA-transpose version lost
    ~18us/step to in-order DMA-queue stalls.)
  - PSUM budget (16 KiB/partition, bank-aligned): two buffers of
    [gates 1792 f32 | 512-col bf16 transpose strip] = 8 KiB each.
  - The whole unroll is ONE software-pipelined stream of (step, layer,
    m-tile) stages: stage s issues [PE transposes of stage s-3] [gate
    matmuls of s] [sig/tanh + c-update of s] [tanh(c) + h-write + DMAs
    of s-2], which keeps the Activation engine (the bottleneck after
    the fp8 matmuls) free of head-of-line stalls.
  - Cell math all-bf16 (2x DVE mode); output DMA reads the packed h1
    slice (out tensor is bf16, upconverted on the host).

kernel(**inputs) takes full-size numpy inputs, packs/shards on the
host, runs the same program SPMD on cores 0..7, and reassembles the
full [8192, 4675] fp32 output (block 0 comes from the initial state).
"""

import numpy as np
import ml_dtypes

BF16 = ml_dtypes.bfloat16
FP8 = ml_dtypes.float8_e4m3

B = 8192
NCORES = 8
BC = B // NCORES          # batch rows per core (1024)
NB = BC // 128            # m-tiles per core (8)
R = 425
IN = 20
GN = 4 * R                # 1700 gate columns
H1OFF = R + 1 + IN        # 446: h1 col offset in the packed state block
HC = 896                  # packed state block width (7*128)
NKC = 7                   # transposed K-chunks holding real data
NK0 = 4                   # layer-0 K-chunks (rows 0..511)
NKT = 11                  # step-0 bf16 weight K-blocks (4 + 7)
NW8 = 12                  # fp8 weight K-blocks (4 + 8, block 11 zero)
NSTEPS = 10
# N chunks of each 850-wide gate half ([i|f] then [o|g]), one PSUM
# bank each
NCHUNKS = [(0, 512), (512, 338)]

# set by test.py to profile; results stashed in LAST_RESULT
TRACE = False
LAST_RESULT = None


def build_bass(n_steps=NSTEPS, finalize=True):
    import concourse.bacc as bacc
    import concourse.mybir as mybir
    import concourse.tile as tile

    f32 = mybir.dt.float32
    bf16 = mybir.dt.bfloat16
    fp8 = mybir.dt.float8e4
    Sig = mybir.ActivationFunctionType.Sigmoid
    Tanh = mybir.ActivationFunctionType.Tanh
    DR = mybir.MatmulPerfMode.DoubleRow

    nc = bacc.Bacc()

    n8 = max(n_steps - 1, 1)
    w0_d = nc.declare_dram_parameter("w0", [128, 7 * GN], bf16, False)
    w8_d = nc.declare_dram_parameter("w8", [n8, 128, NW8 * GN], fp8, False)
    w80_d = nc.declare_dram_parameter("w80", [128, 8 * GN], fp8, False)
    a0_d = nc.declare_dram_parameter("a0", [128, 8 * BC], fp8, False)
    htci_d = nc.declare_dram_parameter("htci", [128, 3 * BC], bf16, False)
    hci_d = nc.declare_dram_parameter("hci", [128, NB * HC], bf16, False)
    c0i_d = nc.declare_dram_parameter("c0i", [128, NB * R], bf16, False)
    c1i_d = nc.declare_dram_parameter("c1i", [128, NB * R], bf16, False)
    eye_d = nc.declare_dram_parameter("eye", [128, 128], bf16, False)
    out_d = nc.declare_dram_parameter("out", [BC, n_steps * R], bf16, True)

    with tile.TileContext(nc) as tc:
        with (
            tc.tile_pool(name="consts", bufs=1) as consts,
            tc.tile_pool(name="wpool", bufs=2) as wpool,
            tc.tile_pool(name="gpsum", bufs=1, space="PSUM") as gpsum,
            tc.tile_pool(name="tmp", bufs=5) as tmp,
        ):
            # persistent state tiles
            hs_t = consts.tile([128, NB * HC], bf16)    # packed batch-major
            htc = consts.tile([128, NKC, BC], bf16)     # bf16 lhsT (t=0 L1)
            htc8 = consts.tile([128, 8, BC], fp8)       # fp8 lhsT (DR)
            c0 = consts.tile([128, NB * R], bf16)
            c1 = consts.tile([128, NB * R], bf16)
            eye = consts.tile([128, 128], bf16)
            w0t = consts.tile([128, 7, GN], bf16)       # t=0 L1 weights
            # step-0 layer-0 fp8 operands: host-packed initial state (+
            # exact 1/8-scaled residual chunks) and its weight blocks
            a0t = consts.tile([128, 8, BC], fp8)
            w80t = consts.tile([128, 8, GN], fp8)

            # PSUM layout (8 banks, nothing shares a bank):
            # 3 rotating gate-half buffers (2 banks each: 850 f32 used)
            # + 2 transpose strips (1 bank each)
            gh0 = gpsum.tile([128, 1024], f32)
            gh1 = gpsum.tile([128, 1024], f32)
            gh2 = gpsum.tile([128, 1024], f32)
            tpA = gpsum.tile([128, 4, 128], bf16)
            tpB = gpsum.tile([128, 4, 128], bf16)
            ghbuf = [gh0, gh1, gh2]
            tpbuf = [tpA, tpB]

            # init DMAs on the SP (HWDGE) queue, quarter-interleaved in
            # conveyor order (stage s consumes c0/hs_t slice s at ~2.3us
            # per stage), while weights stream on the Pool (SWDGE) queue
            nc.sync.dma_start(a0t[:, 0:2, :], a0_d[:, 0:2 * BC])
            nc.sync.dma_start(eye[:], eye_d[:])
            nc.sync.dma_start(c0[:], c0i_d[:])
            for k in range(1, 4):
                nc.sync.dma_start(a0t[:, 2 * k:2 * k + 2, :],
                                  a0_d[:, 2 * k * BC:(2 * k + 2) * BC])
            nc.sync.dma_start(hs_t[:, 0:4 * HC], hci_d[:, 0:4 * HC])
            nc.sync.dma_start(hs_t[:, 4 * HC:], hci_d[:, 4 * HC:])
            for k in range(3):
                nc.sync.dma_start(htc[:, NK0 + k, :],
                                  htci_d[:, k * BC:(k + 1) * BC])
            nc.sync.dma_start(c1[:], c1i_d[:])

            # step-0 weights, split so the first matmuls start early
            for k in range(4):
                nc.gpsimd.dma_start(w80t[:, 2 * k:2 * k + 2, :],
                                    w80_d[:, 2 * k * GN:(2 * k + 2) * GN])
            nc.gpsimd.dma_start(w0t[:], w0_d[:])
            # first fp8 table (for step 1)
            w8tab = {}
            if n_steps > 1:
                w8cur = wpool.tile([128, NW8, GN], fp8, tag="w8")
                for c in range(4):
                    nc.gpsimd.dma_start(w8cur[:, 3 * c:3 * (c + 1), :],
                                        w8_d[0][:, 3 * c * GN:3 * (c + 1) * GN])
                w8tab[1] = w8cur
            # zero pad chunk for layer-1's 4th DoubleRow pass
            nc.gpsimd.memset(htc8[:, 7, :], 0.0)

            # PE warm-up while init DMAs land
            warm = consts.tile([128, 128], bf16)
            nc.vector.memset(warm[:], 0.0)
            for i in range(20):
                nc.tensor.matmul(gh0[:, 0:128], warm[:], warm[:],
                                 start=True, stop=True)

            def mm(gh, t, layer, m):
                """Gate matmuls into the two 850-col PSUM halves."""
                if t == 0:
                    if layer == 0:
                        # 4 DR passes on the host-packed fp8 initial state:
                        # main chunks (0,1),(2,3) then exact-residual
                        # chunks (4,5),(6,7)
                        for ji in range(4):
                            lhsT = a0t[:, 2 * ji:2 * ji + 2,
                                       m * 128:(m + 1) * 128]
                            for h in range(2):
                                for (no, nw) in NCHUNKS:
                                    o = 850 * h + no
                                    nc.tensor.matmul(
                                        gh[h][:, no:no + nw], lhsT,
                                        w80t[:, 2 * ji:2 * ji + 2, o:o + nw],
                                        start=(ji == 0), stop=(ji == 3),
                                        perf_mode=DR)
                    else:
                        # h1 part first (3 bf16 passes on the host-init
                        # transposed state: unbounded h1 + its weights
                        # stay bf16), then the h0' part (4 bf16 passes on
                        # the mid-step transposed device state)
                        kplan = ([(NK0 + k, NK0 + k) for k in range(3)]
                                 + [(k, k) for k in range(NK0)])
                        for ki, (kk, wk) in enumerate(kplan):
                            lhsT = htc[:, kk, m * 128:(m + 1) * 128]
                            for h in range(2):
                                for (no, nw) in NCHUNKS:
                                    o = 850 * h + no
                                    nc.tensor.matmul(
                                        gh[h][:, no:no + nw], lhsT,
                                        w0t[:, wk, o:o + nw],
                                        start=(ki == 0), stop=(ki == 6))
                else:
                    w8t = w8tab[t]
                    jplan = [0, 1] if layer == 0 else [2, 3, 0, 1]
                    nj = len(jplan)
                    for ji, j in enumerate(jplan):
                        wb = 2 * j + (0 if layer == 0 else 4)
                        lhsT = htc8[:, 2 * j:2 * j + 2, m * 128:(m + 1) * 128]
                        for h in range(2):
                            for (no, nw) in NCHUNKS:
                                o = 850 * h + no
                                nc.tensor.matmul(
                                    gh[h][:, no:no + nw], lhsT,
                                    w8t[:, wb:wb + 2, o:o + nw],
                                    start=(ji == 0), stop=(ji == nj - 1),
                                    perf_mode=DR)

            def cell_a(gh, layer, m):
                """Gate nonlinearities + c update (issued in-stage).
                half0 = [i|f], half1 = [o|2g]: the g columns are scaled
                by 2 at weight-pack time so tanh(g) = 2*sigmoid(2g)-1
                and BOTH halves take a single 850-col sigmoid."""
                cst = c0 if layer == 0 else c1
                cs = cst[:, m * R:(m + 1) * R]
                tsig = tmp.tile([128, 2 * R], bf16, tag="tsig")
                nc.scalar.activation(tsig[:], gh[0][:, 0:2 * R], Sig)
                tog = tmp.tile([128, 2 * R], bf16, tag="tog")
                nc.scalar.activation(tog[:], gh[1][:, 0:2 * R], Sig)
                tfc = tmp.tile([128, R], bf16, tag="tfc")
                nc.vector.tensor_mul(tfc[:], tsig[:, R:2 * R], cs)
                # i*tanh(g) = 2*(sig_i*sig_2g) - sig_i
                tp2 = tmp.tile([128, R], bf16, tag="tp2")
                nc.vector.tensor_mul(tp2[:], tsig[:, 0:R], tog[:, R:2 * R])
                tq = tmp.tile([128, R], bf16, tag="tq")
                nc.vector.scalar_tensor_tensor(
                    tq[:], tp2[:], 2.0, tsig[:, 0:R],
                    mybir.AluOpType.mult, mybir.AluOpType.subtract)
                nc.vector.tensor_add(cs, tfc[:], tq[:])
                return tog

            def cell_b2(a, b):
                """tanh(c) + h write + out store for TWO adjacent stages
                (same step+layer, m and m+1): batched 2-row APs halve the
                per-instruction overhead on the ACT/DVE/DMA paths."""
                (tog_a, _, t, layer, m) = a
                (tog_b, _, _, _, m2) = b
                assert m2 == m + 1
                cst = c0 if layer == 0 else c1
                cs2 = cst[:, m * R:(m + 2) * R]
                ttc = tmp.tile([128, 2, R], bf16, tag="ttc")
                nc.scalar.activation(ttc[:], cs2, Tanh)
                off = m * HC + (0 if layer == 0 else H1OFF)
                nc.vector.tensor_mul(hs_t[:, off:off + R],
                                     tog_a[:, 0:R], ttc[:, 0, :])
                nc.vector.tensor_mul(hs_t[:, off + HC:off + HC + R],
                                     tog_b[:, 0:R], ttc[:, 1, :])
                if layer == 1:
                    for mm_ in (m, m + 1):
                        nc.sync.dma_start(
                            out_d[mm_ * 128:(mm_ + 1) * 128,
                                  t * R:(t + 1) * R],
                            hs_t[:, mm_ * HC + H1OFF:mm_ * HC + H1OFF + R])

            def cell_c(tp, t, layer, m):
                """PE-transpose of the freshly written packed-state block
                into the PSUM strip, then one Pool copy into the fp8 lhsT
                (+ a bf16 copy during step 0, whose layer 1 reads bf16).
                Issued 3 stages later so the PE never waits on the DVE."""
                if layer == 0:
                    kk, nk = 0, NK0          # packed cols 0..511
                else:
                    if t >= n_steps - 1:
                        return
                    kk, nk = NK0, NKC - NK0  # packed cols 512..895
                for k in range(nk):
                    nc.tensor.transpose(
                        tp[:, k, :],
                        hs_t[:, m * HC + (kk + k) * 128:
                             m * HC + (kk + k + 1) * 128],
                        eye[:])
                # GPSIMD can't read PSUM: the DVE copies the strip out in
                # bf16 (2x mode) and the idle Pool engine converts to fp8
                nc.vector.tensor_copy(
                    htc[:, kk:kk + nk, m * 128:(m + 1) * 128],
                    tp[:, 0:nk, :])
                nc.gpsimd.tensor_copy(
                    htc8[:, kk:kk + nk, m * 128:(m + 1) * 128],
                    htc[:, kk:kk + nk, m * 128:(m + 1) * 128])

            # ---- the software-pipelined stage stream ----
            stages = [(t, layer, m)
                      for t in range(n_steps)
                      for layer in range(2)
                      for m in range(NB)]
            pend_b = []        # [(tsig, tp, t, layer, m)] awaiting cell_b
            pend_c = []        # [(tp, t, layer, m)] awaiting cell_c
            for s, (t, layer, m) in enumerate(stages):
                if layer == 0 and m == 0 and t >= 1 and t + 1 < n_steps:
                    w8n = wpool.tile([128, NW8, GN], fp8, tag="w8")
                    for c in range(4):
                        nc.gpsimd.dma_start(
                            w8n[:, 3 * c:3 * (c + 1), :],
                            w8_d[t][:, 3 * c * GN:3 * (c + 1) * GN])
                    w8tab[t + 1] = w8n

                if len(pend_c) > 4:
                    cell_c(*pend_c.pop(0))
                gh = (ghbuf[(2 * s) % 3], ghbuf[(2 * s + 1) % 3])
                tp = tpbuf[s % 2]
                mm(gh, t, layer, m)
                tog = cell_a(gh, layer, m)
                if len(pend_b) > 3:
                    a = pend_b.pop(0)
                    b = pend_b.pop(0)
                    cell_b2(a, b)
                pend_b.append((tog, tp, t, layer, m))
                pend_c.append((tp, t, layer, m))
            while pend_b:
                a = pend_b.pop(0)
                b = pend_b.pop(0)
                cell_b2(a, b)
            for args in pend_c:
                cell_c(*args)
    if finalize:
        nc.finalize()
    return nc


def _pack_pf(a):
    """[BC, C] -> [128, NB*C] with m-tile m at cols m*C."""
    c = a.shape[1]
    return np.ascontiguousarray(
        a.reshape(NB, 128, c).transpose(1, 0, 2).reshape(128, NB * c))


def _pack_kt(a):
    """[BC, HC] -> transposed [128, NKC*BC] with K-chunk k at cols k*BC."""
    return np.ascontiguousarray(
        a.T.reshape(NKC, 128, BC).transpose(1, 0, 2).reshape(128, NKC * BC))


def prep_inputs(x, init_states_input, W_i2h0, b_i2h0, W_h2h0, b_h2h0,
                W_i2h1, b_i2h1, W_h2h1, b_h2h1, n_steps=NSTEPS):
    """Host-side packing.  Returns (in_maps, h1_init_full)."""
    x = np.asarray(x, np.float32)
    init = np.asarray(init_states_input, np.float32)
    W_i2h0 = np.asarray(W_i2h0, np.float32)
    b_i2h0 = np.asarray(b_i2h0, np.float32)
    W_h2h0 = np.asarray(W_h2h0, np.float32)
    b_h2h0 = np.asarray(b_h2h0, np.float32)
    W_i2h1 = np.asarray(W_i2h1, np.float32)
    b_i2h1 = np.asarray(b_i2h1, np.float32)
    W_h2h1 = np.asarray(W_h2h1, np.float32)
    b_h2h1 = np.asarray(b_h2h1, np.float32)

    def q8f(a):
        return a.astype(FP8).astype(np.float32)

    # ---- step-0 weight tables ----
    # L0 rows 0..511: [W_h2h0[0].T | b0 | W_i2h0[0].T | 0], g cols x2
    Wl0 = np.zeros((512, GN), np.float32)
    Wl0[0:R] = W_h2h0[0].T
    b0s = b_i2h0[0] + b_h2h0[0]
    Wl0[R] = b0s
    Wl0[R + 1:R + 1 + IN] = W_i2h0[0].T
    Wl0[:, 3 * R:] *= 2.0
    # L1 rows 0..511 (bf16: h0' part + bias + h1 features 0..65)
    Wl1a = np.zeros((512, GN), np.float32)
    Wl1a[0:R] = W_i2h1[0].T
    Wl1a[R] = b_i2h1[0] + b_h2h1[0]
    Wl1a[H1OFF:512] = W_h2h1[0].T[0:512 - H1OFF]
    Wl1a[:, 3 * R:] *= 2.0
    # L1 h1 part rows (packed cols 512..895): W_h2h1[0].T[66..424] + 0
    Wl1h = np.zeros((384, GN), np.float32)
    Wl1h[0:R - (512 - H1OFF)] = W_h2h1[0].T[512 - H1OFF:]
    Wl1h[:, 3 * R:] *= 2.0
    w0_dev = np.ascontiguousarray(
        np.concatenate([Wl1a, Wl1h], axis=0)
        .reshape(7, 128, GN).transpose(1, 0, 2)
        .reshape(128, 7 * GN)).astype(BF16)

    # w80: blocks 0..3 = fp8 main L0, 4..7 = 1/8 residual weights
    W80 = np.zeros((8, 128, GN), np.float32)
    q = q8f(Wl0)
    W80[0:4] = q.reshape(4, 128, GN)
    W80[4:8] = (q / 8).reshape(4, 128, GN)
    W80[7][425 - 3 * 128] = Wl0[R] - q8f(Wl0[R])   # bias residual row
    w80_dev = np.ascontiguousarray(
        W80.transpose(1, 0, 2).reshape(128, 8 * GN)).astype(FP8)

    # ---- fp8 tables for steps 1..n-1 ----
    n8 = max(n_steps - 1, 1)
    Wd8 = np.zeros((n8, NW8 * 128, GN), np.float32)
    for t in range(1, n_steps):
        d = Wd8[t - 1]
        d[0:R] = W_h2h0[t].T
        d[R] = b_i2h0[t] + b_h2h0[t]
        d[R + 1:R + 1 + IN] = W_i2h0[t].T
        o = NK0 * 128
        d[o:o + R] = W_i2h1[t].T
        b1 = b_i2h1[t] + b_h2h1[t]
        d[o + R] = b1
        d[o + H1OFF:o + H1OFF + R] = W_h2h1[t].T
        # residual row (pairs with the 1.0 at packed col 871): cancels
        # most of the fp8 quantization error of the layer-1 bias row
        d[o + H1OFF + R] = b1 - b1.astype(FP8).astype(np.float32)
    Wd8[:, :, 3 * R:] *= 2.0       # g columns: tanh via 2*sig(2x)-1
    w8_dev = np.ascontiguousarray(
        Wd8.reshape(n8, NW8, 128, GN).transpose(0, 2, 1, 3)
        .reshape(n8, 128, NW8 * GN)).astype(FP8)

    init4 = init.reshape(B, 4, R)
    h0_full, c0_full = init4[:, 0], init4[:, 1]
    h1_full, c1_full = init4[:, 2], init4[:, 3]

    eye = np.eye(128, dtype=np.float32).astype(BF16)

    def kmaj8(block, resid_one_col=None):
        """[BC, 512] fp32 -> fp8 [128, 8*BC]: 4 K-major main chunks then
        4 residual chunks (8x the fp8 quantization error)."""
        q = block.astype(FP8).astype(np.float32)
        r8 = 8.0 * (block - q)
        if resid_one_col is not None:
            r8[:, resid_one_col] = 1.0
        both = np.concatenate(
            [q.T.reshape(4, 128, BC), r8.T.reshape(4, 128, BC)], axis=0)
        return np.ascontiguousarray(
            both.transpose(1, 0, 2).reshape(128, 8 * BC)).astype(FP8)

    in_maps = []
    for c in range(NCORES):
        sl = slice(c * BC, (c + 1) * BC)
        hcp = np.zeros((BC, HC), np.float32)
        hcp[:, 0:R] = h0_full[sl]
        hcp[:, R] = 1.0
        hcp[:, R + 1:R + 1 + IN] = x[sl]
        hcp[:, H1OFF:H1OFF + R] = h1_full[sl]
        hcp[:, H1OFF + R] = 1.0
        hcp = hcp.astype(BF16).astype(np.float32)
        in_maps.append({
            "w0": w0_dev,
            "w8": w8_dev,
            "w80": w80_dev,
            "a0": kmaj8(hcp[:, 0:512], resid_one_col=R),
            "htci": np.ascontiguousarray(
                hcp[:, 512:HC].T.reshape(3, 128, BC).transpose(1, 0, 2)
                .reshape(128, 3 * BC)).astype(BF16),
            "hci": _pack_pf(hcp.astype(BF16)),
            "c0i": _pack_pf(np.ascontiguousarray(c0_full[sl])).astype(BF16),
            "c1i": _pack_pf(np.ascontiguousarray(c1_full[sl])).astype(BF16),
            "eye": eye,
        })
    return in_maps, h1_full


def kernel(x, init_states_input, W_i2h0, b_i2h0, W_h2h0, b_h2h0,
           W_i2h1, b_i2h1, W_h2h1, b_h2h1):
    global LAST_RESULT
    from concourse.bass_utils import run_bass_kernel_spmd

    in_maps, h1_full = prep_inputs(
        x, init_states_input, W_i2h0, b_i2h0, W_h2h0, b_h2h0,
        W_i2h1, b_i2h1, W_h2h1, b_h2h1)

    nc = build_bass(NSTEPS)
    res = run_bass_kernel_spmd(nc, in_maps, list(range(NCORES)), trace=TRACE)
    LAST_RESULT = res

    out = np.empty((B, (NSTEPS + 1) * R), np.float32)
    out[:, 0:R] = h1_full
    for c in range(NCORES):
        out[c * BC:(c + 1) * BC, R:] = res.results[c]["out"].astype(np.float32)
    return out


# revision 33
# speedup vs baseline: 1.4574x; 1.0088x over previous
"""Bass/Trainium2 kernel for nn_BuildLstmUnrollNet.

Problem: 2-layer LSTM, unrolled T=11 steps with per-step (non-shared)
weights, B=8192, R=425, IN=20.  Output block t is the last-layer h
*before* step t, so only steps 0..9 need computing.

Strategy (data-parallel over batch, 8 cores x 1024 rows):
  - Step 0 runs in bf16: its matmul operands are the *initial* states,
    which are unbounded N(0,1) draws -- fp8 there costs ~2.5e-2 rel err
    (and the large initial |c| amplifies step-0 gate errors through the
    forget gate).  Steps 1..9 run the gate matmuls in fp8-e4m3
    DoubleRow (both operands fp8, 256-deep contraction per pass): after
    step 0 every h is a tanh*sigmoid product bounded by 1, and e4m3
    keeps the end-to-end rel err ~4e-3 (vs the 2e-2 gate).
  - States kept batch-major in ONE packed bf16 buffer per m-tile:
    cols [h0(425) | 1.0 | x(20) | h1(425) | 1.0 | pad] = 896 = 7*128.
    The second 1.0 (col 871) pairs with a host-precomputed fp8 residual
    weight row that cancels most of layer-1's bias quantization error.
  - Gates are computed batch-major in PSUM with transposed activations
    stationary (lhsT) and weights moving: layer 0 contracts packed rows
    0..511, layer 1 rows 0..1023 (chunk 7 is a zeroed pad chunk so
    layer 1 is exactly 4 DoubleRow passes).
  - The recurrent transpose h -> hT runs on the TENSOR ENGINE
    (transpose-mode matmuls against an identity, 128x128 blocks into a
    PSUM staging strip) and a vector-engine copy moves PSUM -> SBUF with
    the bf16 -> fp8 conversion fused (on the DVE: GPSIMD cannot access
    PSUM).  No DRAM bounce, no DMA-transpose
    queues: the per-m-tile chain is h-write (DVE) -> 3-4 PE transposes
    -> 1 Pool copy, all per-m pipelined.  (A DMA-transpose version lost
    ~18us/step to in-order DMA-queue stalls.)
  - PSUM budget (16 KiB/partition, bank-aligned): two buffers of
    [gates 1792 f32 | 512-col bf16 transpose strip] = 8 KiB each.
  - The whole unroll is ONE software-pipelined stream of (step, layer,
    m-tile) stages: stage s issues [PE transposes of stage s-3] [gate
    matmuls of s] [sig/tanh + c-update of s] [tanh(c) + h-write + DMAs
    of s-2], which keeps the Activation engine (the bottleneck after
    the fp8 matmuls) free of head-of-line stalls.
  - Cell math all-bf16 (2x DVE mode); output DMA reads the packed h1
    slice (out tensor is bf16, upconverted on the host).

kernel(**inputs) takes full-size numpy inputs, packs/shards on the
host, runs the same program SPMD on cores 0..7, and reassembles the
full [8192, 4675] fp32 output (block 0 comes from the initial state).
"""

import numpy as np
import ml_dtypes

BF16 = ml_dtypes.bfloat16
FP8 = ml_dtypes.float8_e4m3

B = 8192
NCORES = 8
BC = B // NCORES          # batch rows per core (1024)
NB = BC // 128            # m-tiles per core (8)
R = 425
IN = 20
GN = 4 * R                # 1700 gate columns
H1OFF = R + 1 + IN        # 446: h1 col offset in the packed state block
HC = 896                  # packed state block width (7*128)
NKC = 7                   # transposed K-chunks holding real data
NK0 = 4                   # layer-0 K-chunks (rows 0..511)
NKT = 11                  # step-0 bf16 weight K-blocks (4 + 7)
NW8 = 12                  # fp8 weight K-blocks (4 + 8, block 11 zero)
NSTEPS = 10
# N chunks of each 850-wide gate half ([i|f] then [o|g]), one PSUM
# bank each
NCHUNKS = [(0, 512), (512, 338)]

# set by test.py to profile; results stashed in LAST_RESULT
TRACE = False
LAST_RESULT = None


def build_bass(n_steps=NSTEPS, finalize=True):
    import concourse.bacc as bacc
    import concourse.mybir as mybir
    import concourse.tile as tile

    f32 = mybir.dt.float32
    bf16 = mybir.dt.bfloat16
    fp8 = mybir.dt.float8e4
    Sig = mybir.ActivationFunctionType.Sigmoid
    Tanh = mybir.ActivationFunctionType.Tanh
    DR = mybir.MatmulPerfMode.DoubleRow

    nc = bacc.Bacc()

    n8 = max(n_steps - 1, 1)
    w0_d = nc.declare_dram_parameter("w0", [128, 7 * GN], bf16, False)
    w8_d = nc.declare_dram_parameter("w8", [n8, 128, NW8 * GN], fp8, False)
    w80_d = nc.declare_dram_parameter("w80", [128, 8 * GN], fp8, False)
    a0_d = nc.declare_dram_parameter("a0", [128, 8 * BC], fp8, False)
    htci_d = nc.declare_dram_parameter("htci", [128, 3 * BC], bf16, False)
    hci_d = nc.declare_dram_parameter("hci", [128, NB * HC], bf16, False)
    c0i_d = nc.declare_dram_parameter("c0i", [128, NB * R], bf16, False)
    c1i_d = nc.declare_dram_parameter("c1i", [128, NB * R], bf16, False)
    eye_d = nc.declare_dram_parameter("eye", [128, 128], bf16, False)
    out_d = nc.declare_dram_parameter("out", [BC, n_steps * R], bf16, True)

    with tile.TileContext(nc) as tc:
        with (
            tc.tile_pool(name="consts", bufs=1) as consts,
            tc.tile_pool(name="wpool", bufs=2) as wpool,
            tc.tile_pool(name="gpsum", bufs=1, space="PSUM") as gpsum,
            tc.tile_pool(name="tmp", bufs=5) as tmp,
        ):
            # persistent state tiles
            hs_t = consts.tile([128, NB * HC], bf16)    # packed batch-major
            htc = consts.tile([128, NKC, BC], bf16)     # bf16 lhsT (t=0 L1)
            htc8 = consts.tile([128, 8, BC], fp8)       # fp8 lhsT (DR)
            c0 = consts.tile([128, NB * R], bf16)
            c1 = consts.tile([128, NB * R], bf16)
            eye = consts.tile([128, 128], bf16)
            w0t = consts.tile([128, 7, GN], bf16)       # t=0 L1 weights
            # step-0 layer-0 fp8 operands: host-packed initial state (+
            # exact 1/8-scaled residual chunks) and its weight blocks
            a0t = consts.tile([128, 8, BC], fp8)
            w80t = consts.tile([128, 8, GN], fp8)

            # PSUM layout (8 banks, nothing shares a bank):
            # 3 rotating gate-half buffers (2 banks each: 850 f32 used)
            # + 2 transpose strips (1 bank each)
            gh0 = gpsum.tile([128, 1024], f32)
            gh1 = gpsum.tile([128, 1024], f32)
            gh2 = gpsum.tile([128, 1024], f32)
            tpA = gpsum.tile([128, 4, 128], bf16)
            tpB = gpsum.tile([128, 4, 128], bf16)
            ghbuf = [gh0, gh1, gh2]
            tpbuf = [tpA, tpB]

            # init DMAs on the SP (HWDGE) queue, quarter-interleaved in
            # conveyor order (stage s consumes c0/hs_t slice s at ~2.3us
            # per stage), while weights stream on the Pool (SWDGE) queue
            nc.sync.dma_start(a0t[:, 0:2, :], a0_d[:, 0:2 * BC])
            nc.sync.dma_start(eye[:], eye_d[:])
            for k in range(1, 4):
                nc.sync.dma_start(a0t[:, 2 * k:2 * k + 2, :],
                                  a0_d[:, 2 * k * BC:(2 * k + 2) * BC])
            nc.sync.dma_start(c0[:], c0i_d[:])
            nc.sync.dma_start(hs_t[:, 0:4 * HC], hci_d[:, 0:4 * HC])
            nc.sync.dma_start(hs_t[:, 4 * HC:], hci_d[:, 4 * HC:])
            for k in range(3):
                nc.sync.dma_start(htc[:, NK0 + k, :],
                                  htci_d[:, k * BC:(k + 1) * BC])
            nc.sync.dma_start(c1[:], c1i_d[:])

            # step-0 weights, split so the first matmuls start early
            for k in range(4):
                nc.gpsimd.dma_start(w80t[:, 2 * k:2 * k + 2, :],
                                    w80_d[:, 2 * k * GN:(2 * k + 2) * GN])
            # h1-part blocks (4..6) first: layer 1's first passes read
            # them ~25us in; the h0'-part blocks follow
            nc.gpsimd.dma_start(w0t[:, 4:7, :], w0_d[:, 4 * GN:])
            nc.gpsimd.dma_start(w0t[:, 0:4, :], w0_d[:, 0:4 * GN])
            # first fp8 table (for step 1)
            w8tab = {}
            if n_steps > 1:
                w8cur = wpool.tile([128, NW8, GN], fp8, tag="w8")
                for c in range(4):
                    nc.gpsimd.dma_start(w8cur[:, 3 * c:3 * (c + 1), :],
                                        w8_d[0][:, 3 * c * GN:3 * (c + 1) * GN])
                w8tab[1] = w8cur
            # zero pad chunk for layer-1's 4th DoubleRow pass
            nc.gpsimd.memset(htc8[:, 7, :], 0.0)

            # PE warm-up while init DMAs land
            warm = consts.tile([128, 128], bf16)
            nc.vector.memset(warm[:], 0.0)
            for i in range(20):
                nc.tensor.matmul(gh0[:, 0:128], warm[:], warm[:],
                                 start=True, stop=True)

            def mm(gh, t, layer, m):
                """Gate matmuls into the two 850-col PSUM halves."""
                if t == 0:
                    if layer == 0:
                        # 4 DR passes on the host-packed fp8 initial state:
                        # main chunks (0,1),(2,3) then exact-residual
                        # chunks (4,5),(6,7)
                        for ji in range(4):
                            lhsT = a0t[:, 2 * ji:2 * ji + 2,
                                       m * 128:(m + 1) * 128]
                            for h in range(2):
                                for (no, nw) in NCHUNKS:
                                    o = 850 * h + no
                                    nc.tensor.matmul(
                                        gh[h][:, no:no + nw], lhsT,
                                        w80t[:, 2 * ji:2 * ji + 2, o:o + nw],
                                        start=(ji == 0), stop=(ji == 3),
                                        perf_mode=DR)
                    else:
                        # h1 part first (3 bf16 passes on the host-init
                        # transposed state: unbounded h1 + its weights
                        # stay bf16), then the h0' part (4 bf16 passes on
                        # the mid-step transposed device state)
                        kplan = ([(NK0 + k, NK0 + k) for k in range(3)]
                                 + [(k, k) for k in range(NK0)])
                        for ki, (kk, wk) in enumerate(kplan):
                            lhsT = htc[:, kk, m * 128:(m + 1) * 128]
                            for h in range(2):
                                for (no, nw) in NCHUNKS:
                                    o = 850 * h + no
                                    nc.tensor.matmul(
                                        gh[h][:, no:no + nw], lhsT,
                                        w0t[:, wk, o:o + nw],
                                        start=(ki == 0), stop=(ki == 6))
                else:
                    w8t = w8tab[t]
                    jplan = [0, 1] if layer == 0 else [2, 3, 0, 1]
                    nj = len(jplan)
                    for ji, j in enumerate(jplan):
                        wb = 2 * j + (0 if layer == 0 else 4)
                        lhsT = htc8[:, 2 * j:2 * j + 2, m * 128:(m + 1) * 128]
                        for h in range(2):
                            for (no, nw) in NCHUNKS:
                                o = 850 * h + no
                                nc.tensor.matmul(
                                    gh[h][:, no:no + nw], lhsT,
                                    w8t[:, wb:wb + 2, o:o + nw],
                                    start=(ji == 0), stop=(ji == nj - 1),
                                    perf_mode=DR)

            def cell_a(gh, layer, m):
                """Gate nonlinearities + c update (issued in-stage).
                half0 = [i|f], half1 = [o|2g]: the g columns are scaled
                by 2 at weight-pack time so tanh(g) = 2*sigmoid(2g)-1
                and BOTH halves take a single 850-col sigmoid."""
                cst = c0 if layer == 0 else c1
                cs = cst[:, m * R:(m + 1) * R]
                tsig = tmp.tile([128, 2 * R], bf16, tag="tsig")
                nc.scalar.activation(tsig[:], gh[0][:, 0:2 * R], Sig)
                tog = tmp.tile([128, 2 * R], bf16, tag="tog")
                nc.scalar.activation(tog[:], gh[1][:, 0:2 * R], Sig)
                tfc = tmp.tile([128, R], bf16, tag="tfc")
                nc.vector.tensor_mul(tfc[:], tsig[:, R:2 * R], cs)
                # i*tanh(g) = 2*(sig_i*sig_2g) - sig_i
                tp2 = tmp.tile([128, R], bf16, tag="tp2")
                nc.vector.tensor_mul(tp2[:], tsig[:, 0:R], tog[:, R:2 * R])
                tq = tmp.tile([128, R], bf16, tag="tq")
                nc.vector.scalar_tensor_tensor(
                    tq[:], tp2[:], 2.0, tsig[:, 0:R],
                    mybir.AluOpType.mult, mybir.AluOpType.subtract)
                nc.vector.tensor_add(cs, tfc[:], tq[:])
                return tog

            def cell_b2(a, b):
                """tanh(c) + h write + out store for TWO adjacent stages
                (same step+layer, m and m+1): batched 2-row APs halve the
                per-instruction overhead on the ACT/DVE/DMA paths."""
                (tog_a, _, t, layer, m) = a
                (tog_b, _, _, _, m2) = b
                assert m2 == m + 1
                cst = c0 if layer == 0 else c1
                cs2 = cst[:, m * R:(m + 2) * R]
                ttc = tmp.tile([128, 2, R], bf16, tag="ttc")
                nc.scalar.activation(ttc[:], cs2, Tanh)
                off = m * HC + (0 if layer == 0 else H1OFF)
                nc.vector.tensor_mul(hs_t[:, off:off + R],
                                     tog_a[:, 0:R], ttc[:, 0, :])
                nc.vector.tensor_mul(hs_t[:, off + HC:off + HC + R],
                                     tog_b[:, 0:R], ttc[:, 1, :])
                if layer == 1:
                    for mm_ in (m, m + 1):
                        nc.sync.dma_start(
                            out_d[mm_ * 128:(mm_ + 1) * 128,
                                  t * R:(t + 1) * R],
                            hs_t[:, mm_ * HC + H1OFF:mm_ * HC + H1OFF + R])

            def cell_c(tp, t, layer, m):
                """PE-transpose of the freshly written packed-state block
                into the PSUM strip, then one Pool copy into the fp8 lhsT
                (+ a bf16 copy during step 0, whose layer 1 reads bf16).
                Issued 3 stages later so the PE never waits on the DVE."""
                if layer == 0:
                    kk, nk = 0, NK0          # packed cols 0..511
                else:
                    if t >= n_steps - 1:
                        return
                    kk, nk = NK0, NKC - NK0  # packed cols 512..895
                for k in range(nk):
                    nc.tensor.transpose(
                        tp[:, k, :],
                        hs_t[:, m * HC + (kk + k) * 128:
                             m * HC + (kk + k + 1) * 128],
                        eye[:])
                # GPSIMD can't read PSUM: the DVE copies the strip out in
                # bf16 (2x mode) and the idle Pool engine converts to fp8
                nc.vector.tensor_copy(
                    htc[:, kk:kk + nk, m * 128:(m + 1) * 128],
                    tp[:, 0:nk, :])
                nc.gpsimd.tensor_copy(
                    htc8[:, kk:kk + nk, m * 128:(m + 1) * 128],
                    htc[:, kk:kk + nk, m * 128:(m + 1) * 128])

            # ---- the software-pipelined stage stream ----
            stages = [(t, layer, m)
                      for t in range(n_steps)
                      for layer in range(2)
                      for m in range(NB)]
            pend_b = []        # [(tsig, tp, t, layer, m)] awaiting cell_b
            pend_c = []        # [(tp, t, layer, m)] awaiting cell_c
            for s, (t, layer, m) in enumerate(stages):
                if layer == 0 and m == 0 and t >= 1 and t + 1 < n_steps:
                    w8n = wpool.tile([128, NW8, GN], fp8, tag="w8")
                    for c in range(4):
                        nc.gpsimd.dma_start(
                            w8n[:, 3 * c:3 * (c + 1), :],
                            w8_d[t][:, 3 * c * GN:3 * (c + 1) * GN])
                    w8tab[t + 1] = w8n

                if len(pend_c) > 4:
                    cell_c(*pend_c.pop(0))
                gh = (ghbuf[(2 * s) % 3], ghbuf[(2 * s + 1) % 3])
                tp = tpbuf[s % 2]
                mm(gh, t, layer, m)
                tog = cell_a(gh, layer, m)
                if len(pend_b) > 3:
                    a = pend_b.pop(0)
                    b = pend_b.pop(0)
                    cell_b2(a, b)
                pend_b.append((tog, tp, t, layer, m))
                pend_c.append((tp, t, layer, m))
            while pend_b:
                a = pend_b.pop(0)
                b = pend_b.pop(0)
                cell_b2(a, b)
            for args in pend_c:
                cell_c(*args)
    if finalize:
        nc.finalize()
    return nc


def _pack_pf(a):
    """[BC, C] -> [128, NB*C] with m-tile m at cols m*C."""
    c = a.shape[1]
    return np.ascontiguousarray(
        a.reshape(NB, 128, c).transpose(1, 0, 2).reshape(128, NB * c))


def _pack_kt(a):
    """[BC, HC] -> transposed [128, NKC*BC] with K-chunk k at cols k*BC."""
    return np.ascontiguousarray(
        a.T.reshape(NKC, 128, BC).transpose(1, 0, 2).reshape(128, NKC * BC))


def prep_inputs(x, init_states_input, W_i2h0, b_i2h0, W_h2h0, b_h2h0,
                W_i2h1, b_i2h1, W_h2h1, b_h2h1, n_steps=NSTEPS):
    """Host-side packing.  Returns (in_maps, h1_init_full)."""
    x = np.asarray(x, np.float32)
    init = np.asarray(init_states_input, np.float32)
    W_i2h0 = np.asarray(W_i2h0, np.float32)
    b_i2h0 = np.asarray(b_i2h0, np.float32)
    W_h2h0 = np.asarray(W_h2h0, np.float32)
    b_h2h0 = np.asarray(b_h2h0, np.float32)
    W_i2h1 = np.asarray(W_i2h1, np.float32)
    b_i2h1 = np.asarray(b_i2h1, np.float32)
    W_h2h1 = np.asarray(W_h2h1, np.float32)
    b_h2h1 = np.asarray(b_h2h1, np.float32)

    def q8f(a):
        return a.astype(FP8).astype(np.float32)

    # ---- step-0 weight tables ----
    # L0 rows 0..511: [W_h2h0[0].T | b0 | W_i2h0[0].T | 0], g cols x2
    Wl0 = np.zeros((512, GN), np.float32)
    Wl0[0:R] = W_h2h0[0].T
    b0s = b_i2h0[0] + b_h2h0[0]
    Wl0[R] = b0s
    Wl0[R + 1:R + 1 + IN] = W_i2h0[0].T
    Wl0[:, 3 * R:] *= 2.0
    # L1 rows 0..511 (bf16: h0' part + bias + h1 features 0..65)
    Wl1a = np.zeros((512, GN), np.float32)
    Wl1a[0:R] = W_i2h1[0].T
    Wl1a[R] = b_i2h1[0] + b_h2h1[0]
    Wl1a[H1OFF:512] = W_h2h1[0].T[0:512 - H1OFF]
    Wl1a[:, 3 * R:] *= 2.0
    # L1 h1 part rows (packed cols 512..895): W_h2h1[0].T[66..424] + 0
    Wl1h = np.zeros((384, GN), np.float32)
    Wl1h[0:R - (512 - H1OFF)] = W_h2h1[0].T[512 - H1OFF:]
    Wl1h[:, 3 * R:] *= 2.0
    w0_dev = np.ascontiguousarray(
        np.concatenate([Wl1a, Wl1h], axis=0)
        .reshape(7, 128, GN).transpose(1, 0, 2)
        .reshape(128, 7 * GN)).astype(BF16)

    # w80: blocks 0..3 = fp8 main L0, 4..7 = 1/8 residual weights
    W80 = np.zeros((8, 128, GN), np.float32)
    q = q8f(Wl0)
    W80[0:4] = q.reshape(4, 128, GN)
    W80[4:8] = (q / 8).reshape(4, 128, GN)
    W80[7][425 - 3 * 128] = Wl0[R] - q8f(Wl0[R])   # bias residual row
    w80_dev = np.ascontiguousarray(
        W80.transpose(1, 0, 2).reshape(128, 8 * GN)).astype(FP8)

    # ---- fp8 tables for steps 1..n-1 ----
    n8 = max(n_steps - 1, 1)
    Wd8 = np.zeros((n8, NW8 * 128, GN), np.float32)
    for t in range(1, n_steps):
        d = Wd8[t - 1]
        d[0:R] = W_h2h0[t].T
        d[R] = b_i2h0[t] + b_h2h0[t]
        d[R + 1:R + 1 + IN] = W_i2h0[t].T
        o = NK0 * 128
        d[o:o + R] = W_i2h1[t].T
        b1 = b_i2h1[t] + b_h2h1[t]
        d[o + R] = b1
        d[o + H1OFF:o + H1OFF + R] = W_h2h1[t].T
        # residual row (pairs with the 1.0 at packed col 871): cancels
        # most of the fp8 quantization error of the layer-1 bias row
        d[o + H1OFF + R] = b1 - b1.astype(FP8).astype(np.float32)
    Wd8[:, :, 3 * R:] *= 2.0       # g columns: tanh via 2*sig(2x)-1
    w8_dev = np.ascontiguousarray(
        Wd8.reshape(n8, NW8, 128, GN).transpose(0, 2, 1, 3)
        .reshape(n8, 128, NW8 * GN)).astype(FP8)

    init4 = init.reshape(B, 4, R)
    h0_full, c0_full = init4[:, 0], init4[:, 1]
    h1_full, c1_full = init4[:, 2], init4[:, 3]

    eye = np.eye(128, dtype=np.float32).astype(BF16)

    def kmaj8(block, resid_one_col=None):
        """[BC, 512] fp32 -> fp8 [128, 8*BC]: 4 K-major main chunks then
        4 residual chunks (8x the fp8 quantization error)."""
        q = block.astype(FP8).astype(np.float32)
        r8 = 8.0 * (block - q)
        if resid_one_col is not None:
            r8[:, resid_one_col] = 1.0
        both = np.concatenate(
            [q.T.reshape(4, 128, BC), r8.T.reshape(4, 128, BC)], axis=0)
        return np.ascontiguousarray(
            both.transpose(1, 0, 2).reshape(128, 8 * BC)).astype(FP8)

    in_maps = []
    for c in range(NCORES):
        sl = slice(c * BC, (c + 1) * BC)
        hcp = np.zeros((BC, HC), np.float32)
        hcp[:, 0:R] = h0_full[sl]
        hcp[:, R] = 1.0
        hcp[:, R + 1:R + 1 + IN] = x[sl]
        hcp[:, H1OFF:H1OFF + R] = h1_full[sl]
        hcp[:, H1OFF + R] = 1.0
        hcp = hcp.astype(BF16).astype(np.float32)
        in_maps.append({
            "w0": w0_dev,
            "w8": w8_dev,
            "w80": w80_dev,
            "a0": kmaj8(hcp[:, 0:512], resid_one_col=R),
            "htci": np.ascontiguousarray(
                hcp[:, 512:HC].T.reshape(3, 128, BC).transpose(1, 0, 2)
                .reshape(128, 3 * BC)).astype(BF16),
            "hci": _pack_pf(hcp.astype(BF16)),
            "c0i": _pack_pf(np.ascontiguousarray(c0_full[sl])).astype(BF16),
            "c1i": _pack_pf(np.ascontiguousarray(c1_full[sl])).astype(BF16),
            "eye": eye,
        })
    return in_maps, h1_full


def kernel(x, init_states_input, W_i2h0, b_i2h0, W_h2h0, b_h2h0,
           W_i2h1, b_i2h1, W_h2h1, b_h2h1):
    global LAST_RESULT
    from concourse.bass_utils import run_bass_kernel_spmd

    in_maps, h1_full = prep_inputs(
        x, init_states_input, W_i2h0, b_i2h0, W_h2h0, b_h2h0,
        W_i2h1, b_i2h1, W_h2h1, b_h2h1)

    nc = build_bass(NSTEPS)
    res = run_bass_kernel_spmd(nc, in_maps, list(range(NCORES)), trace=TRACE)
    LAST_RESULT = res

    out = np.empty((B, (NSTEPS + 1) * R), np.float32)
    out[:, 0:R] = h1_full
    for c in range(NCORES):
        out[c * BC:(c + 1) * BC, R:] = res.results[c]["out"].astype(np.float32)
    return out


# revision 35
# speedup vs baseline: 1.4736x; 1.0111x over previous
"""Bass/Trainium2 kernel for nn_BuildLstmUnrollNet.

Problem: 2-layer LSTM, unrolled T=11 steps with per-step (non-shared)
weights, B=8192, R=425, IN=20.  Output block t is the last-layer h
*before* step t, so only steps 0..9 need computing.

Strategy (data-parallel over batch, 8 cores x 1024 rows):
  - Step 0 runs in bf16: its matmul operands are the *initial* states,
    which are unbounded N(0,1) draws -- fp8 there costs ~2.5e-2 rel err
    (and the large initial |c| amplifies step-0 gate errors through the
    forget gate).  Steps 1..9 run the gate matmuls in fp8-e4m3
    DoubleRow (both operands fp8, 256-deep contraction per pass): after
    step 0 every h is a tanh*sigmoid product bounded by 1, and e4m3
    keeps the end-to-end rel err ~4e-3 (vs the 2e-2 gate).
  - States kept batch-major in ONE packed bf16 buffer per m-tile:
    cols [h0(425) | 1.0 | x(20) | h1(425) | 1.0 | pad] = 896 = 7*128.
    The second 1.0 (col 871) pairs with a host-precomputed fp8 residual
    weight row that cancels most of layer-1's bias quantization error.
  - Gates are computed batch-major in PSUM with transposed activations
    stationary (lhsT) and weights moving: layer 0 contracts packed rows
    0..511, layer 1 rows 0..1023 (chunk 7 is a zeroed pad chunk so
    layer 1 is exactly 4 DoubleRow passes).
  - The recurrent transpose h -> hT runs on the TENSOR ENGINE
    (transpose-mode matmuls against an identity, 128x128 blocks into a
    PSUM staging strip) and a vector-engine copy moves PSUM -> SBUF with
    the bf16 -> fp8 conversion fused (on the DVE: GPSIMD cannot access
    PSUM).  No DRAM bounce, no DMA-transpose
    queues: the per-m-tile chain is h-write (DVE) -> 3-4 PE transposes
    -> 1 Pool copy, all per-m pipelined.  (A DMA-transpose version lost
    ~18us/step to in-order DMA-queue stalls.)
  - PSUM budget (16 KiB/partition, bank-aligned): two buffers of
    [gates 1792 f32 | 512-col bf16 transpose strip] = 8 KiB each.
  - The whole unroll is ONE software-pipelined stream of (step, layer,
    m-tile) stages: stage s issues [PE transposes of stage s-3] [gate
    matmuls of s] [sig/tanh + c-update of s] [tanh(c) + h-write + DMAs
    of s-2], which keeps the Activation engine (the bottleneck after
    the fp8 matmuls) free of head-of-line stalls.
  - Cell math all-bf16 (2x DVE mode); output DMA reads the packed h1
    slice (out tensor is bf16, upconverted on the host).

kernel(**inputs) takes full-size numpy inputs, packs/shards on the
host, runs the same program SPMD on cores 0..7, and reassembles the
full [8192, 4675] fp32 output (block 0 comes from the initial state).
"""

import numpy as np
import ml_dtypes

BF16 = ml_dtypes.bfloat16
FP8 = ml_dtypes.float8_e4m3

B = 8192
NCORES = 8
BC = B // NCORES          # batch rows per core (1024)
NB = BC // 128            # m-tiles per core (8)
R = 425
IN = 20
GN = 4 * R                # 1700 gate columns
H1OFF = R + 1 + IN        # 446: h1 col offset in the packed state block
HC = 896                  # packed state block width (7*128)
NKC = 7                   # transposed K-chunks holding real data
NK0 = 4                   # layer-0 K-chunks (rows 0..511)
NKT = 11                  # step-0 bf16 weight K-blocks (4 + 7)
NW8 = 12                  # fp8 weight K-blocks (4 + 8, block 11 zero)
NSTEPS = 10
# N chunks of each 850-wide gate half ([i|f] then [o|g]), one PSUM
# bank each
NCHUNKS = [(0, 512), (512, 338)]

# set by test.py to profile; results stashed in LAST_RESULT
TRACE = False
LAST_RESULT = None


def build_bass(n_steps=NSTEPS, finalize=True):
    import concourse.bacc as bacc
    import concourse.mybir as mybir
    import concourse.tile as tile

    f32 = mybir.dt.float32
    bf16 = mybir.dt.bfloat16
    fp8 = mybir.dt.float8e4
    Sig = mybir.ActivationFunctionType.Sigmoid
    Tanh = mybir.ActivationFunctionType.Tanh
    DR = mybir.MatmulPerfMode.DoubleRow

    nc = bacc.Bacc()

    n8 = max(n_steps - 1, 1)
    w0_d = nc.declare_dram_parameter("w0", [128, 4 * GN], bf16, False)
    w8_d = nc.declare_dram_parameter("w8", [n8, 128, NW8 * GN], fp8, False)
    w80_d = nc.declare_dram_parameter("w80", [128, 16 * GN], fp8, False)
    a0_d = nc.declare_dram_parameter("a0", [128, 8 * BC], fp8, False)
    htci_d = nc.declare_dram_parameter("htci", [128, 3 * BC], bf16, False)
    hci_d = nc.declare_dram_parameter("hci", [128, NB * HC], bf16, False)
    c0i_d = nc.declare_dram_parameter("c0i", [128, NB * R], bf16, False)
    c1i_d = nc.declare_dram_parameter("c1i", [128, NB * R], bf16, False)
    eye_d = nc.declare_dram_parameter("eye", [128, 128], bf16, False)
    out_d = nc.declare_dram_parameter("out", [BC, n_steps * R], bf16, True)

    with tile.TileContext(nc) as tc:
        with (
            tc.tile_pool(name="consts", bufs=1) as consts,
            tc.tile_pool(name="wpool", bufs=2) as wpool,
            tc.tile_pool(name="gpsum", bufs=1, space="PSUM") as gpsum,
            tc.tile_pool(name="tmp", bufs=5) as tmp,
        ):
            # persistent state tiles
            hs_t = consts.tile([128, NB * HC], bf16)    # packed batch-major
            htc = consts.tile([128, NKC, BC], bf16)     # bf16 lhsT (t=0 L1)
            htc8 = consts.tile([128, 8, BC], fp8)       # fp8 lhsT (DR)
            c0 = consts.tile([128, NB * R], bf16)
            c1 = consts.tile([128, NB * R], bf16)
            eye = consts.tile([128, 128], bf16)
            w0t = consts.tile([128, 4, GN], bf16)       # t=0 L1 h1 weights
            # step-0 layer-0 fp8 operands: host-packed initial state (+
            # exact 1/8-scaled residual chunks) and its weight blocks
            a0t = consts.tile([128, 8, BC], fp8)
            w80t = consts.tile([128, 16, GN], fp8)

            # PSUM layout (8 banks, nothing shares a bank):
            # 3 rotating gate-half buffers (2 banks each: 850 f32 used)
            # + 2 transpose strips (1 bank each)
            gh0 = gpsum.tile([128, 1024], f32)
            gh1 = gpsum.tile([128, 1024], f32)
            gh2 = gpsum.tile([128, 1024], f32)
            tpA = gpsum.tile([128, 4, 128], bf16)
            tpB = gpsum.tile([128, 4, 128], bf16)
            ghbuf = [gh0, gh1, gh2]
            tpbuf = [tpA, tpB]

            # init DMAs on the SP (HWDGE) queue, quarter-interleaved in
            # conveyor order (stage s consumes c0/hs_t slice s at ~2.3us
            # per stage), while weights stream on the Pool (SWDGE) queue
            nc.sync.dma_start(a0t[:, 0:2, :], a0_d[:, 0:2 * BC])
            nc.sync.dma_start(eye[:], eye_d[:])
            for k in range(1, 4):
                nc.sync.dma_start(a0t[:, 2 * k:2 * k + 2, :],
                                  a0_d[:, 2 * k * BC:(2 * k + 2) * BC])
            nc.sync.dma_start(c0[:], c0i_d[:])
            nc.sync.dma_start(hs_t[:, 0:4 * HC], hci_d[:, 0:4 * HC])
            nc.sync.dma_start(hs_t[:, 4 * HC:], hci_d[:, 4 * HC:])
            for k in range(3):
                nc.sync.dma_start(htc[:, NK0 + k, :],
                                  htci_d[:, k * BC:(k + 1) * BC])
            nc.sync.dma_start(c1[:], c1i_d[:])

            # step-0 weights, split so the first matmuls start early
            for k in range(4):
                nc.gpsimd.dma_start(w80t[:, 2 * k:2 * k + 2, :],
                                    w80_d[:, 2 * k * GN:(2 * k + 2) * GN])
            nc.gpsimd.dma_start(w0t[:], w0_d[:])
            for k in range(4, 8):
                nc.gpsimd.dma_start(w80t[:, 2 * k:2 * k + 2, :],
                                    w80_d[:, 2 * k * GN:(2 * k + 2) * GN])
            # first fp8 table (for step 1)
            w8tab = {}
            if n_steps > 1:
                w8cur = wpool.tile([128, NW8, GN], fp8, tag="w8")
                for c in range(4):
                    nc.gpsimd.dma_start(w8cur[:, 3 * c:3 * (c + 1), :],
                                        w8_d[0][:, 3 * c * GN:3 * (c + 1) * GN])
                w8tab[1] = w8cur
            # zero pad chunk for layer-1's 4th DoubleRow pass
            nc.gpsimd.memset(htc8[:, 7, :], 0.0)

            # PE warm-up while init DMAs land
            warm = consts.tile([128, 128], bf16)
            nc.vector.memset(warm[:], 0.0)
            for i in range(20):
                nc.tensor.matmul(gh0[:, 0:128], warm[:], warm[:],
                                 start=True, stop=True)

            def mm(gh, t, layer, m):
                """Gate matmuls into the two 850-col PSUM halves."""
                if t == 0:
                    if layer == 0:
                        # 4 DR passes on the host-packed fp8 initial state:
                        # main chunks (0,1),(2,3) then exact-residual
                        # chunks (4,5),(6,7)
                        for ji in range(4):
                            lhsT = a0t[:, 2 * ji:2 * ji + 2,
                                       m * 128:(m + 1) * 128]
                            for h in range(2):
                                for (no, nw) in NCHUNKS:
                                    o = 850 * h + no
                                    nc.tensor.matmul(
                                        gh[h][:, no:no + nw], lhsT,
                                        w80t[:, 2 * ji:2 * ji + 2, o:o + nw],
                                        start=(ji == 0), stop=(ji == 3),
                                        perf_mode=DR)
                    else:
                        # h1 features 66..424 first (3 bf16 passes on the
                        # host-init transposed state: unbounded values stay
                        # bf16), then the h0' part (4 DR passes: fp8 main +
                        # 1/8-scale weight residuals on the same device
                        # chunks), and last a bf16 chunk-3 pass carrying
                        # the bias and h1 features 0..65
                        for ki in range(3):
                            lhsT = htc[:, NK0 + ki, m * 128:(m + 1) * 128]
                            for h in range(2):
                                for (no, nw) in NCHUNKS:
                                    o = 850 * h + no
                                    nc.tensor.matmul(
                                        gh[h][:, no:no + nw], lhsT,
                                        w0t[:, 1 + ki, o:o + nw],
                                        start=(ki == 0), stop=False)
                        for ji in range(4):
                            wb = 8 + 2 * (ji % 2)
                            if ji >= 2:
                                wb += 4          # residual blocks 12..15
                            lhsT = htc8[:, 2 * (ji % 2):2 * (ji % 2) + 2,
                                        m * 128:(m + 1) * 128]
                            for h in range(2):
                                for (no, nw) in NCHUNKS:
                                    o = 850 * h + no
                                    nc.tensor.matmul(
                                        gh[h][:, no:no + nw], lhsT,
                                        w80t[:, wb:wb + 2, o:o + nw],
                                        start=False, stop=False,
                                        perf_mode=DR)
                        lhsT = htc[:, 3, m * 128:(m + 1) * 128]
                        for h in range(2):
                            for (no, nw) in NCHUNKS:
                                o = 850 * h + no
                                nc.tensor.matmul(
                                    gh[h][:, no:no + nw], lhsT,
                                    w0t[:, 0, o:o + nw],
                                    start=False, stop=True)
                else:
                    w8t = w8tab[t]
                    jplan = [0, 1] if layer == 0 else [2, 3, 0, 1]
                    nj = len(jplan)
                    for ji, j in enumerate(jplan):
                        wb = 2 * j + (0 if layer == 0 else 4)
                        lhsT = htc8[:, 2 * j:2 * j + 2, m * 128:(m + 1) * 128]
                        for h in range(2):
                            for (no, nw) in NCHUNKS:
                                o = 850 * h + no
                                nc.tensor.matmul(
                                    gh[h][:, no:no + nw], lhsT,
                                    w8t[:, wb:wb + 2, o:o + nw],
                                    start=(ji == 0), stop=(ji == nj - 1),
                                    perf_mode=DR)

            def cell_a(gh, layer, m):
                """Gate nonlinearities + c update (issued in-stage).
                half0 = [i|f], half1 = [o|2g]: the g columns are scaled
                by 2 at weight-pack time so tanh(g) = 2*sigmoid(2g)-1
                and BOTH halves take a single 850-col sigmoid."""
                cst = c0 if layer == 0 else c1
                cs = cst[:, m * R:(m + 1) * R]
                tsig = tmp.tile([128, 2 * R], bf16, tag="tsig")
                nc.scalar.activation(tsig[:], gh[0][:, 0:2 * R], Sig)
                tog = tmp.tile([128, 2 * R], bf16, tag="tog")
                nc.scalar.activation(tog[:], gh[1][:, 0:2 * R], Sig)
                tfc = tmp.tile([128, R], bf16, tag="tfc")
                nc.vector.tensor_mul(tfc[:], tsig[:, R:2 * R], cs)
                # i*tanh(g) = 2*(sig_i*sig_2g) - sig_i
                tp2 = tmp.tile([128, R], bf16, tag="tp2")
                nc.vector.tensor_mul(tp2[:], tsig[:, 0:R], tog[:, R:2 * R])
                tq = tmp.tile([128, R], bf16, tag="tq")
                nc.vector.scalar_tensor_tensor(
                    tq[:], tp2[:], 2.0, tsig[:, 0:R],
                    mybir.AluOpType.mult, mybir.AluOpType.subtract)
                nc.vector.tensor_add(cs, tfc[:], tq[:])
                return tog

            def cell_b2(a, b):
                """tanh(c) + h write + out store for TWO adjacent stages
                (same step+layer, m and m+1): batched 2-row APs halve the
                per-instruction overhead on the ACT/DVE/DMA paths."""
                (tog_a, _, t, layer, m) = a
                (tog_b, _, _, _, m2) = b
                assert m2 == m + 1
                cst = c0 if layer == 0 else c1
                cs2 = cst[:, m * R:(m + 2) * R]
                ttc = tmp.tile([128, 2, R], bf16, tag="ttc")
                nc.scalar.activation(ttc[:], cs2, Tanh)
                off = m * HC + (0 if layer == 0 else H1OFF)
                nc.vector.tensor_mul(hs_t[:, off:off + R],
                                     tog_a[:, 0:R], ttc[:, 0, :])
                nc.vector.tensor_mul(hs_t[:, off + HC:off + HC + R],
                                     tog_b[:, 0:R], ttc[:, 1, :])
                if layer == 1:
                    for mm_ in (m, m + 1):
                        nc.sync.dma_start(
                            out_d[mm_ * 128:(mm_ + 1) * 128,
                                  t * R:(t + 1) * R],
                            hs_t[:, mm_ * HC + H1OFF:mm_ * HC + H1OFF + R])

            def cell_c(tp, t, layer, m):
                """PE-transpose of the freshly written packed-state block
                into the PSUM strip, then one Pool copy into the fp8 lhsT
                (+ a bf16 copy during step 0, whose layer 1 reads bf16).
                Issued 3 stages later so the PE never waits on the DVE."""
                if layer == 0:
                    kk, nk = 0, NK0          # packed cols 0..511
                else:
                    if t >= n_steps - 1:
                        return
                    kk, nk = NK0, NKC - NK0  # packed cols 512..895
                for k in range(nk):
                    nc.tensor.transpose(
                        tp[:, k, :],
                        hs_t[:, m * HC + (kk + k) * 128:
                             m * HC + (kk + k + 1) * 128],
                        eye[:])
                # GPSIMD can't read PSUM: the DVE copies the strip out in
                # bf16 (2x mode) and the idle Pool engine converts to fp8
                if t == 0 and layer == 0:
                    # step-0 L1 reads the fp8 chunks directly; only the
                    # chunk-3 consts/h1-head pass needs a bf16 copy
                    nc.vector.tensor_copy(
                        htc8[:, kk:kk + nk, m * 128:(m + 1) * 128],
                        tp[:, 0:nk, :])
                    nc.vector.tensor_copy(
                        htc[:, 3, m * 128:(m + 1) * 128], tp[:, 3, :])
                else:
                    nc.vector.tensor_copy(
                        htc[:, kk:kk + nk, m * 128:(m + 1) * 128],
                        tp[:, 0:nk, :])
                    nc.gpsimd.tensor_copy(
                        htc8[:, kk:kk + nk, m * 128:(m + 1) * 128],
                        htc[:, kk:kk + nk, m * 128:(m + 1) * 128])

            # ---- the software-pipelined stage stream ----
            stages = [(t, layer, m)
                      for t in range(n_steps)
                      for layer in range(2)
                      for m in range(NB)]
            pend_b = []        # [(tsig, tp, t, layer, m)] awaiting cell_b
            pend_c = []        # [(tp, t, layer, m)] awaiting cell_c
            for s, (t, layer, m) in enumerate(stages):
                if layer == 0 and m == 0 and t >= 1 and t + 1 < n_steps:
                    w8n = wpool.tile([128, NW8, GN], fp8, tag="w8")
                    for c in range(4):
                        nc.gpsimd.dma_start(
                            w8n[:, 3 * c:3 * (c + 1), :],
                            w8_d[t][:, 3 * c * GN:3 * (c + 1) * GN])
                    w8tab[t + 1] = w8n

                if len(pend_c) > 4:
                    cell_c(*pend_c.pop(0))
                gh = (ghbuf[(2 * s) % 3], ghbuf[(2 * s + 1) % 3])
                tp = tpbuf[s % 2]
                mm(gh, t, layer, m)
                tog = cell_a(gh, layer, m)
                if len(pend_b) > 3:
                    a = pend_b.pop(0)
                    b = pend_b.pop(0)
                    cell_b2(a, b)
                pend_b.append((tog, tp, t, layer, m))
                pend_c.append((tp, t, layer, m))
            while pend_b:
                a = pend_b.pop(0)
                b = pend_b.pop(0)
                cell_b2(a, b)
            for args in pend_c:
                cell_c(*args)
    if finalize:
        nc.finalize()
    return nc


def _pack_pf(a):
    """[BC, C] -> [128, NB*C] with m-tile m at cols m*C."""
    c = a.shape[1]
    return np.ascontiguousarray(
        a.reshape(NB, 128, c).transpose(1, 0, 2).reshape(128, NB * c))


def _pack_kt(a):
    """[BC, HC] -> transposed [128, NKC*BC] with K-chunk k at cols k*BC."""
    return np.ascontiguousarray(
        a.T.reshape(NKC, 128, BC).transpose(1, 0, 2).reshape(128, NKC * BC))


def prep_inputs(x, init_states_input, W_i2h0, b_i2h0, W_h2h0, b_h2h0,
                W_i2h1, b_i2h1, W_h2h1, b_h2h1, n_steps=NSTEPS):
    """Host-side packing.  Returns (in_maps, h1_init_full)."""
    x = np.asarray(x, np.float32)
    init = np.asarray(init_states_input, np.float32)
    W_i2h0 = np.asarray(W_i2h0, np.float32)
    b_i2h0 = np.asarray(b_i2h0, np.float32)
    W_h2h0 = np.asarray(W_h2h0, np.float32)
    b_h2h0 = np.asarray(b_h2h0, np.float32)
    W_i2h1 = np.asarray(W_i2h1, np.float32)
    b_i2h1 = np.asarray(b_i2h1, np.float32)
    W_h2h1 = np.asarray(W_h2h1, np.float32)
    b_h2h1 = np.asarray(b_h2h1, np.float32)

    def q8f(a):
        return a.astype(FP8).astype(np.float32)

    # ---- step-0 weight tables ----
    # L0 rows 0..511: [W_h2h0[0].T | b0 | W_i2h0[0].T | 0], g cols x2
    Wl0 = np.zeros((512, GN), np.float32)
    Wl0[0:R] = W_h2h0[0].T
    b0s = b_i2h0[0] + b_h2h0[0]
    Wl0[R] = b0s
    Wl0[R + 1:R + 1 + IN] = W_i2h0[0].T
    Wl0[:, 3 * R:] *= 2.0
    # bf16 L1 table (4 blocks): block 0 pairs the device chunk 3
    # (bias row + h1 features 0..65; the h0' rows there are handled by
    # the fp8 blocks, so zero); blocks 1..3 pair chunks 4..6 (h1
    # features 66..424)
    Wl1_c3 = np.zeros((128, GN), np.float32)
    Wl1_c3[R - 384] = b_i2h1[0] + b_h2h1[0]
    Wl1_c3[H1OFF - 384:] = W_h2h1[0].T[0:512 - H1OFF]
    Wl1h = np.zeros((384, GN), np.float32)
    Wl1h[0:R - (512 - H1OFF)] = W_h2h1[0].T[512 - H1OFF:]
    w0blk = np.concatenate([Wl1_c3, Wl1h], axis=0)
    w0blk[:, 3 * R:] *= 2.0
    w0_dev = np.ascontiguousarray(
        w0blk.reshape(4, 128, GN).transpose(1, 0, 2)
        .reshape(128, 4 * GN)).astype(BF16)

    # w80: blocks 0..3 = fp8 main L0, 4..7 = 1/8 residual weights,
    # 8..11 = fp8 L1-h0' main (W_i2h1 rows only), 12..15 = its 1/8
    # residuals (same act chunks as the main passes)
    W80 = np.zeros((16, 128, GN), np.float32)
    q = q8f(Wl0)
    W80[0:4] = q.reshape(4, 128, GN)
    W80[4:8] = (q / 8).reshape(4, 128, GN)
    W80[7][425 - 3 * 128] = Wl0[R] - q8f(Wl0[R])   # bias residual row
    Wl1a = np.zeros((512, GN), np.float32)
    Wl1a[0:R] = W_i2h1[0].T
    Wl1a[:, 3 * R:] *= 2.0
    qa = q8f(Wl1a)
    W80[8:12] = qa.reshape(4, 128, GN)
    W80[12:16] = (q8f(8 * (Wl1a - qa)) / 8).reshape(4, 128, GN)
    w80_dev = np.ascontiguousarray(
        W80.transpose(1, 0, 2).reshape(128, 16 * GN)).astype(FP8)

    # ---- fp8 tables for steps 1..n-1 ----
    n8 = max(n_steps - 1, 1)
    Wd8 = np.zeros((n8, NW8 * 128, GN), np.float32)
    for t in range(1, n_steps):
        d = Wd8[t - 1]
        d[0:R] = W_h2h0[t].T
        d[R] = b_i2h0[t] + b_h2h0[t]
        d[R + 1:R + 1 + IN] = W_i2h0[t].T
        o = NK0 * 128
        d[o:o + R] = W_i2h1[t].T
        b1 = b_i2h1[t] + b_h2h1[t]
        d[o + R] = b1
        d[o + H1OFF:o + H1OFF + R] = W_h2h1[t].T
        # residual row (pairs with the 1.0 at packed col 871): cancels
        # most of the fp8 quantization error of the layer-1 bias row
        d[o + H1OFF + R] = b1 - b1.astype(FP8).astype(np.float32)
    Wd8[:, :, 3 * R:] *= 2.0       # g columns: tanh via 2*sig(2x)-1
    w8_dev = np.ascontiguousarray(
        Wd8.reshape(n8, NW8, 128, GN).transpose(0, 2, 1, 3)
        .reshape(n8, 128, NW8 * GN)).astype(FP8)

    init4 = init.reshape(B, 4, R)
    h0_full, c0_full = init4[:, 0], init4[:, 1]
    h1_full, c1_full = init4[:, 2], init4[:, 3]

    eye = np.eye(128, dtype=np.float32).astype(BF16)

    def kmaj8(block, resid_one_col=None):
        """[BC, 512] fp32 -> fp8 [128, 8*BC]: 4 K-major main chunks then
        4 residual chunks (8x the fp8 quantization error)."""
        q = block.astype(FP8).astype(np.float32)
        r8 = 8.0 * (block - q)
        if resid_one_col is not None:
            r8[:, resid_one_col] = 1.0
        both = np.concatenate(
            [q.T.reshape(4, 128, BC), r8.T.reshape(4, 128, BC)], axis=0)
        return np.ascontiguousarray(
            both.transpose(1, 0, 2).reshape(128, 8 * BC)).astype(FP8)

    in_maps = []
    for c in range(NCORES):
        sl = slice(c * BC, (c + 1) * BC)
        hcp = np.zeros((BC, HC), np.float32)
        hcp[:, 0:R] = h0_full[sl]
        hcp[:, R] = 1.0
        hcp[:, R + 1:R + 1 + IN] = x[sl]
        hcp[:, H1OFF:H1OFF + R] = h1_full[sl]
        hcp[:, H1OFF + R] = 1.0
        hcp = hcp.astype(BF16).astype(np.float32)
        in_maps.append({
            "w0": w0_dev,
            "w8": w8_dev,
            "w80": w80_dev,
            "a0": kmaj8(hcp[:, 0:512], resid_one_col=R),
            "htci": np.ascontiguousarray(
                hcp[:, 512:HC].T.reshape(3, 128, BC).transpose(1, 0, 2)
                .reshape(128, 3 * BC)).astype(BF16),
            "hci": _pack_pf(hcp.astype(BF16)),
            "c0i": _pack_pf(np.ascontiguousarray(c0_full[sl])).astype(BF16),
            "c1i": _pack_pf(np.ascontiguousarray(c1_full[sl])).astype(BF16),
            "eye": eye,
        })
    return in_maps, h1_full


def kernel(x, init_states_input, W_i2h0, b_i2h0, W_h2h0, b_h2h0,
           W_i2h1, b_i2h1, W_h2h1, b_h2h1):
    global LAST_RESULT
    from concourse.bass_utils import run_bass_kernel_spmd

    in_maps, h1_full = prep_inputs(
        x, init_states_input, W_i2h0, b_i2h0, W_h2h0, b_h2h0,
        W_i2h1, b_i2h1, W_h2h1, b_h2h1)

    nc = build_bass(NSTEPS)
    res = run_bass_kernel_spmd(nc, in_maps, list(range(NCORES)), trace=TRACE)
    LAST_RESULT = res

    out = np.empty((B, (NSTEPS + 1) * R), np.float32)
    out[:, 0:R] = h1_full
    for c in range(NCORES):
        out[c * BC:(c + 1) * BC, R:] = res.results[c]["out"].astype(np.float32)
    return out
